# revision 1
# baseline (speedup 1.0000x reference)
"""Distributed causal self-attention for 8 TRN2 NeuronCores.

Sharding: tensor-parallel over heads (2 heads/core, all batches), then an
on-device AllToAll redistributes the attention output from head-sharded to
token-sharded so each core computes a disjoint 1024-token slice of the
output projection.  Host work is only slicing / concatenation.

Layouts (per core g, heads h0=2g, h1=2g+1), all matmuls bf16 with fp32 PSUM:
  qT/kT  [128, BT]    rows 0:64 = head h0 dims, 64:128 = head h1 dims
  v_sb   [128, 130*n] per 128-token chunk: cols [130c:130c+64] = v_h0,
                      col 130c+64 = ones, [130c+65:130c+129] = v_h1,
                      col 130c+129 = ones (ones give the softmax sums)
  S^T    [k, q] in PSUM; exp on ACT (no max-subtraction needed: scores are
         ~N(0,1), |S|<~8 after 1/sqrt(D) scaling, exp never overflows fp32);
         causal mask = bf16 0/1 multiply on GpSimd
  O^T    accumulated in PSUM via matmul(lhsT=v_ext[128,65], rhs=P^T) so
         row 64 = sum_k exp = softmax denominator

Engine budget: PE = matmuls, ACT = exp only, DVE = PSUM->SBUF copies +
normalization, GpSimd = causal masks + collectives, SP = DMA.
The AllToAll is split in two (by 512-token window parity) so the first
collective overlaps the second half of attention and the second overlaps
the first half of the output projection.
"""

import numpy as np

import concourse.bass as bass
import concourse.bacc as bacc
import concourse.mybir as mybir
import concourse.tile as tile
from concourse.bass_utils import run_bass_kernel_spmd

B, T, C = 4, 2048, 1024
H, D = 16, 64
NCORES = 8
HPC = H // NCORES        # heads per core
DH = HPC * D             # 128 attention-output cols per core
P = 128
F32 = mybir.dt.float32
BF16 = mybir.dt.bfloat16
SCALE = 1.0 / np.sqrt(D)


def build_nc(Tb=T, reps=1, stages="ABC", skip_collective=False, bvariant="full", pt_bufs=10, seq_a=True, ps_bufs=3, po_bufs=2, small_bufs=3):
    """Build the SPMD Bass graph (identical on all 8 cores).

    reps > 1 emits the whole pipeline that many times (same buffers, so
    iterations serialize) — used only for steady-state HW timing.
    """
    BT = B * Tb              # total tokens
    NTW = BT // 512          # 512-token windows for QKV
    NQW = Tb // 512          # query windows per batch
    NCH = BT // 128          # 128-token chunks total
    TOKS = BT // NCORES      # tokens per core in the proj stage
    NNW = C // 512           # 512-wide output column windows
    NPH = 2 if Tb >= 2048 else 1   # A2A phase count (split needs 512 | TOKS/NPH)
    HTOK = TOKS // NPH       # tokens per proj phase (A2A split)

    nc = bacc.Bacc(None, target_bir_lowering=False)

    xT_ext = nc.declare_dram_parameter("xT", [C, BT], BF16, isOutput=False)
    wq_ext = nc.declare_dram_parameter("wq", [C, DH], BF16, isOutput=False)
    wk_ext = nc.declare_dram_parameter("wk", [C, DH], BF16, isOutput=False)
    wv_ext = nc.declare_dram_parameter("wv", [C, DH], BF16, isOutput=False)
    wp_ext = nc.declare_dram_parameter("wproj", [C, C], BF16, isOutput=False)
    mk_ext = nc.declare_dram_parameter("masks", [4, P, 512], BF16, isOutput=False)
    id_ext = nc.declare_dram_parameter("ident", [P, P], BF16, isOutput=False)
    y_ext = nc.declare_dram_parameter("y", [TOKS, C], F32, isOutput=True)

    xT_v = xT_ext.rearrange("(c p) t -> p c t", p=P)     # [128, 8, BT]
    wq_v = wq_ext.rearrange("(c p) m -> p c m", p=P)     # [128, 8, 128]
    wk_v = wk_ext.rearrange("(c p) m -> p c m", p=P)
    wv_v = wv_ext.rearrange("(c p) m -> p c m", p=P)
    wp_v = wp_ext.rearrange("(c p) m -> p c m", p=P)     # [128, 8, 1024]
    mk_v = mk_ext.rearrange("j p t -> p j t")            # [128, 4, 512]

    with tile.TileContext(nc, num_cores=NCORES) as tc:
        with (
            tc.tile_pool(name="consts", bufs=1) as consts,
            tc.tile_pool(name="acts", bufs=1) as acts,
            tc.tile_pool(name="xin", bufs=2) as xin,
            tc.tile_pool(name="small", bufs=small_bufs) as small,
            tc.tile_pool(name="ptiles", bufs=pt_bufs) as ptiles,
            tc.tile_pool(name="psum", bufs=1, space="PSUM") as psum,
            tc.tile_pool(name="dram", bufs=1, space="DRAM") as dram,
        ):
            # ---- constants ----
            wq_sb = consts.tile([P, 8, DH], BF16)
            wk_sb = consts.tile([P, 8, DH], BF16)
            wv_sb = consts.tile([P, 8, DH], BF16)
            wp_sb = consts.tile([P, 8, C], BF16)
            mk_sb = consts.tile([P, 4, 512], BF16)
            id_sb = consts.tile([P, P], BF16)
            ones_sb = consts.tile([1, P], BF16)
            nc.gpsimd.dma_start(wq_sb[:], wq_v[:])
            nc.gpsimd.dma_start(wk_sb[:], wk_v[:])
            nc.gpsimd.dma_start(wv_sb[:], wv_v[:])
            nc.gpsimd.dma_start(wp_sb[:], wp_v[:])
            nc.gpsimd.dma_start(mk_sb[:], mk_v[:])
            nc.gpsimd.dma_start(id_sb[:], id_ext[:])
            nc.vector.memset(ones_sb[:], 1.0)

            # ---- persistent activations ----
            qT_sb = acts.tile([P, BT], BF16)
            kT_sb = acts.tile([P, BT], BF16)
            v_sb = acts.tile([P, 130 * NCH], BF16)
            nc.vector.memset(v_sb[:], 1.0)  # bakes in the ones columns

            a2a_in = [dram.tile([NCORES, P, HTOK], BF16, name=f"a2ain{p}",
                                tag=f"a2ain{p}") for p in range(NPH)]
            a2a_out = [dram.tile([NCORES, P, HTOK], BF16, name=f"a2aout{p}",
                                 tag=f"a2aout{p}") for p in range(NPH)]

            for rep in range(reps):
                # ================= Stage A: QKV projection =================
                for tw in range(NTW):
                    xw = xin.tile([P, 8, 512], BF16, tag="xw")
                    nc.sync.dma_start(xw[:], xT_v[:, :, 512 * tw : 512 * (tw + 1)])
                    if seq_a:
                        pq = psum.tile([P, 512], F32, tag="stA", bufs=2)
                        for cc in range(8):
                            nc.tensor.matmul(pq[:], wq_sb[:, cc, :], xw[:, cc, :],
                                             start=(cc == 0), stop=(cc == 7))
                        nc.vector.tensor_copy(qT_sb[:, 512 * tw : 512 * (tw + 1)], pq[:])
                        pk = psum.tile([P, 512], F32, tag="stA", bufs=2)
                        for cc in range(8):
                            nc.tensor.matmul(pk[:], wk_sb[:, cc, :], xw[:, cc, :],
                                             start=(cc == 0), stop=(cc == 7))
                        nc.vector.tensor_copy(kT_sb[:, 512 * tw : 512 * (tw + 1)], pk[:])
                        pvT = psum.tile([P, 512], F32, tag="stA", bufs=2)
                        for cc in range(8):
                            nc.tensor.matmul(pvT[:], wv_sb[:, cc, :], xw[:, cc, :],
                                             start=(cc == 0), stop=(cc == 7))
                        vT_tmp = small.tile([P, 512], BF16, tag="vT")
                        nc.vector.tensor_copy(vT_tmp[:], pvT[:])
                    else:
                        pq = psum.tile([P, 512], F32, tag="pq", bufs=1)
                        pk = psum.tile([P, 512], F32, tag="pk", bufs=1)
                        pvT = psum.tile([P, 512], F32, tag="pvT", bufs=1)
                        for cc in range(8):
                            st, sp = (cc == 0), (cc == 7)
                            rhs = xw[:, cc, :]
                            nc.tensor.matmul(pq[:], wq_sb[:, cc, :], rhs,
                                             start=st, stop=sp)
                            nc.tensor.matmul(pk[:], wk_sb[:, cc, :], rhs,
                                             start=st, stop=sp)
                            nc.tensor.matmul(pvT[:], wv_sb[:, cc, :], rhs,
                                             start=st, stop=sp)
                        nc.vector.tensor_copy(qT_sb[:, 512 * tw : 512 * (tw + 1)], pq[:])
                        nc.vector.tensor_copy(kT_sb[:, 512 * tw : 512 * (tw + 1)], pk[:])
                        vT_tmp = small.tile([P, 512], BF16, tag="vT")
                        nc.vector.tensor_copy(vT_tmp[:], pvT[:])
                    for j in range(4):
                        pv = psum.tile([P, P], BF16, tag="aux", bufs=1)
                        nc.tensor.transpose(pv[:], vT_tmp[:, P * j : P * (j + 1)],
                                            id_sb[:])
                        gc = 4 * tw + j
                        # both heads in one strided copy: cols {0:64, 65:129}
                        dst = v_sb[:, 130 * gc : 130 * gc + 130].rearrange(
                            "p (h d) -> p h d", h=2, d=65)[:, :, 0:64]
                        src = pv[:, :].rearrange("p (h d) -> p h d", h=2)
                        nc.vector.tensor_copy(dst, src)

                # ================= Stage B: causal attention =================
                # qw-major order so each A2A phase's inputs finish early.
                if "B" not in stages:
                    continue
                phases = [(p, [qw for qw in range(NQW)
                               if ((512 * qw) % TOKS) // HTOK == p])
                          for p in range(NPH)]
                for phase, qws in phases:
                    for qw in qws:
                        for b in range(B):
                            for lh in range(HPC):
                                hs = 64 * lh
                                q0 = Tb * b + 512 * qw
                                kmax = 4 * qw + 4
                                po = psum.tile([P, 512], F32, tag="po", bufs=po_bufs)
                                for kc in range(kmax):
                                    k0 = Tb * b + P * kc
                                    j = kc - 4 * qw
                                    # diagonal tiles: columns [0 : 128j] are
                                    # fully causal-masked -> skip them entirely
                                    c0 = max(0, j) * P
                                    ps = psum.tile([P, 512], F32, tag="ps", bufs=ps_bufs)
                                    nc.tensor.matmul(
                                        ps[:, c0:512],
                                        kT_sb[hs : hs + 64, k0 : k0 + P],
                                        qT_sb[hs : hs + 64, q0 + c0 : q0 + 512],
                                        start=True, stop=True)
                                    if bvariant == "sonly":
                                        continue
                                    pT = ptiles.tile([P, 512], BF16, tag="pT")
                                    func = (mybir.ActivationFunctionType.Copy
                                            if bvariant == "noexp" else
                                            mybir.ActivationFunctionType.Exp)
                                    nc.scalar.activation(
                                        pT[:, c0:512], ps[:, c0:512],
                                        func, scale=float(SCALE))
                                    if j >= 0:
                                        nc.vector.tensor_mul(pT[:, c0:512],
                                                             pT[:, c0:512],
                                                             mk_sb[:, j, c0:512])
                                    if bvariant == "nopv":
                                        continue
                                    gc = (Tb // 128) * b + kc
                                    nc.tensor.matmul(
                                        po[0:65, c0:512],
                                        v_sb[:, 130 * gc + 65 * lh
                                             : 130 * gc + 65 * lh + 65],
                                        pT[:, c0:512],
                                        start=(kc == 0), stop=(kc == kmax - 1),
                                        skip_group_check=True)
                                # normalize: rows 0:64 / row 64
                                if bvariant in ("sonly", "nopv", "nonorm"):
                                    continue
                                rec = small.tile([1, 512], F32, tag="rec")
                                nc.vector.reciprocal(rec[:], po[64:65, :])
                                rec_bf = small.tile([1, 512], BF16, tag="recb")
                                nc.vector.tensor_copy(rec_bf[:], rec[:])
                                pb = psum.tile([P, 512], F32, tag="aux", bufs=1)
                                nc.tensor.matmul(pb[:], ones_sb[:], rec_bf[:],
                                                 start=True, stop=True)
                                rb = small.tile([P, 512], F32, tag="rb")
                                nc.vector.tensor_copy(rb[:], pb[:])
                                ao = small.tile([64, 512], BF16, tag="ao")
                                nc.vector.tensor_mul(ao[:], po[0:64, :],
                                                     rb[0:64, :])
                                # dest: global tokens [q0 : q0+512], split on
                                # rank/phase chunk boundaries
                                seg = 0
                                while seg < 512:
                                    t = q0 + seg
                                    r, off = t // TOKS, t % TOKS
                                    ph, offp = off // HTOK, off % HTOK
                                    ln = min(512 - seg, HTOK - offp)
                                    nc.sync.dma_start(
                                        a2a_in[ph][r, hs : hs + 64,
                                                   offp : offp + ln],
                                        ao[:, seg : seg + ln])
                                    seg += ln
                    # fire this phase's AllToAll
                    if skip_collective:
                        continue
                    nc.gpsimd.collective_compute(
                        "AllToAll",
                        mybir.AluOpType.bypass,
                        replica_groups=[list(range(NCORES))],
                        ins=[a2a_in[phase].opt()],
                        outs=[a2a_out[phase].opt()],
                    )

                # ================= Stage C: output projection =================
                if "C" not in stages:
                    continue
                for phase in range(NPH):
                    ga = acts.tile([P, 8, HTOK], BF16, name=f"ga{phase}",
                                   tag=f"ga{phase}")
                    nc.sync.dma_start(ga[:],
                                      a2a_out[phase].rearrange("j p t -> p j t"))
                    for tc2 in range(HTOK // P):
                        for nw in range(NNW):
                            py = psum.tile([P, 512], F32, tag="ps", bufs=ps_bufs)
                            for cc in range(8):
                                nc.tensor.matmul(
                                    py[:],
                                    ga[:, cc, P * tc2 : P * (tc2 + 1)],
                                    wp_sb[:, cc, 512 * nw : 512 * (nw + 1)],
                                    start=(cc == 0), stop=(cc == 7))
                            ys = small.tile([P, 512], F32, tag="ys")
                            nc.vector.tensor_copy(ys[:], py[:])
                            nc.sync.dma_start(
                                y_ext[HTOK * phase + P * tc2
                                      : HTOK * phase + P * (tc2 + 1),
                                      512 * nw : 512 * (nw + 1)],
                                ys[:])

    nc.finalize()
    return nc


def _host_inputs(x, w_attn, w_proj, Tb=T):
    import ml_dtypes
    bf16 = ml_dtypes.bfloat16
    BT = B * Tb
    xT = np.ascontiguousarray(x.reshape(BT, C).T).astype(bf16)
    wproj_bf = np.ascontiguousarray(w_proj).astype(bf16)
    jj = np.arange(4)[:, None, None]
    rr = np.arange(P)[None, :, None]
    ccols = np.arange(512)[None, None, :]
    masks = (P * jj + rr <= ccols).astype(bf16)
    ident = np.eye(P).astype(bf16)
    in_maps = []
    for g in range(NCORES):
        in_maps.append({
            "xT": xT,
            "wq": np.ascontiguousarray(w_attn[:, DH * g : DH * (g + 1)]).astype(bf16),
            "wk": np.ascontiguousarray(w_attn[:, C + DH * g : C + DH * (g + 1)]).astype(bf16),
            "wv": np.ascontiguousarray(w_attn[:, 2 * C + DH * g : 2 * C + DH * (g + 1)]).astype(bf16),
            "wproj": wproj_bf,
            "masks": masks,
            "ident": ident,
        })
    return in_maps


_NC_CACHE = {}


def kernel(x, w_attn, w_proj):
    x = np.asarray(x)
    w_attn = np.asarray(w_attn)
    w_proj = np.asarray(w_proj)
    if T not in _NC_CACHE:
        _NC_CACHE[T] = build_nc(T)
    nc = _NC_CACHE[T]
    in_maps = _host_inputs(x, w_attn, w_proj, T)
    res = run_bass_kernel_spmd(nc, in_maps, core_ids=list(range(NCORES)))
    y = np.concatenate([res.results[g]["y"] for g in range(NCORES)], axis=0)
    return y.reshape(B, T, C).astype(np.float32)



# revision 22
# speedup vs baseline: 1.0982x; 1.0982x over previous
"""Distributed causal self-attention for 8 TRN2 NeuronCores.

Sharding: tensor-parallel over heads (2 heads/core, all batches), then an
on-device AllToAll redistributes the attention output from head-sharded to
token-sharded so each core computes a disjoint 1024-token slice of the
output projection.  Host work is only slicing / concatenation.

v2 pipeline design (vs the v1 staged design):
  - QKV, attention and output projection are emitted in one interleaved
    stream ordered by token windows, so ACT(exp) starts almost immediately
    and every engine stays busy.
  - Q and K projections accumulate into one [128, 2, 512] PSUM pair tile
    (one DVE copy); V is computed directly in [token, dim] layout (no PE
    transposes, no identity matrix).
  - Both heads' S^T tiles live in one [128, 2, 512] PSUM pair -> ONE exp
    activation per chunk; optional tile_position row-packing runs the two
    64-contraction S matmuls concurrently in the PE array halves.
  - The causal mask multiply only touches the 128-wide triangular block of
    each diagonal chunk (columns beyond it are unmasked) and runs on the
    otherwise-idle GPSIMD(Pool) engine.
  - Softmax normalization: reciprocal (DVE) -> partition_broadcast (GPSIMD)
    -> one tensor_mul (DVE).  No PE matmul in the normalization chain, so
    the PE stream is pure matmuls.
  - The phase-0 output projection is interleaved into the late attention
    groups; only the phase-1 A2A + projection is a tail.

Engine budget: PE = matmuls only, ACT = exp only (the stage-B bound),
DVE = PSUM->SBUF copies + reciprocal + normalize mul, GpSimd = masks +
broadcast + collectives, SP = DMA.
"""

import numpy as np

import concourse.bass as bass
import concourse.bacc as bacc
import concourse.mybir as mybir
import concourse.tile as tile
from concourse import library_config
from concourse.bass_utils import run_bass_kernel_spmd

B, T, C = 4, 2048, 1024
H, D = 16, 64
NCORES = 8
HPC = H // NCORES        # heads per core
DH = HPC * D             # 128 attention-output cols per core
P = 128
F32 = mybir.dt.float32
BF16 = mybir.dt.bfloat16
SCALE = 1.0 / np.sqrt(D)


def build_nc(Tb=T, reps=1, stages="ABC", skip_collective=False, bvariant="full",
             pack_s=True, use_pbcast=True, mask_pool=True, pair_bufs=2,
             po_bufs=3, pt_bufs=8):
    """Build the SPMD Bass graph (identical on all 8 cores).

    reps > 1 emits the whole pipeline that many times (same buffers, so
    iterations serialize) -- used only for steady-state HW timing.
    """
    BT = B * Tb              # total tokens
    NTW = BT // 512          # 512-token windows
    NQW = Tb // 512          # query windows per batch
    NCH = BT // 128          # 128-token chunks total
    TOKS = BT // NCORES      # tokens per core in the proj stage
    NNW = C // 512           # 512-wide output column windows
    NPH = 2 if Tb >= 2048 else 1
    HTOK = TOKS // NPH       # tokens per proj phase (A2A split)

    nc = bacc.Bacc(None, target_bir_lowering=False)

    xT_ext = nc.declare_dram_parameter("xT", [C, BT], BF16, isOutput=False)
    wq_ext = nc.declare_dram_parameter("wq", [C, DH], BF16, isOutput=False)
    wk_ext = nc.declare_dram_parameter("wk", [C, DH], BF16, isOutput=False)
    wv_ext = nc.declare_dram_parameter("wv", [C, DH], BF16, isOutput=False)
    wp_ext = nc.declare_dram_parameter("wproj", [C, C], BF16, isOutput=False)
    mk_ext = nc.declare_dram_parameter("masks", [P, 2 * P], BF16, isOutput=False)
    y_ext = nc.declare_dram_parameter("y", [TOKS, C], F32, isOutput=True)

    xT_v = xT_ext.rearrange("(c p) t -> p c t", p=P)     # [128, 8, BT]
    wq_v = wq_ext.rearrange("(c p) m -> p c m", p=P)     # [128, 8, 128]
    wk_v = wk_ext.rearrange("(c p) m -> p c m", p=P)
    wv_v = wv_ext.rearrange("(c p) m -> p c m", p=P)
    wp_v = wp_ext.rearrange("(c p) m -> p c m", p=P)     # [128, 8, 1024]

    with tile.TileContext(nc, num_cores=NCORES) as tc:
        with (
            tc.tile_pool(name="consts", bufs=1) as consts,
            tc.tile_pool(name="acts", bufs=1) as acts,
            tc.tile_pool(name="xin", bufs=2) as xin,
            tc.tile_pool(name="small", bufs=4) as small,
            tc.tile_pool(name="ptiles", bufs=pt_bufs) as ptiles,
            tc.tile_pool(name="psum", bufs=1, space="PSUM") as psum,
            tc.tile_pool(name="dram", bufs=1, space="DRAM") as dram,
        ):
            if use_pbcast:
                nc.gpsimd.load_library(library_config.proxy)

            # ---- constants ----
            wq_sb = consts.tile([P, 8, DH], BF16)
            wk_sb = consts.tile([P, 8, DH], BF16)
            wv_sb = consts.tile([P, 8, DH], BF16)
            wp_sb = consts.tile([P, 8, C], BF16)
            mk_sb = consts.tile([P, 2, P], BF16)
            nc.gpsimd.dma_start(wq_sb[:], wq_v[:])
            nc.gpsimd.dma_start(wk_sb[:], wk_v[:])
            nc.gpsimd.dma_start(wv_sb[:], wv_v[:])
            nc.gpsimd.dma_start(wp_sb[:], wp_v[:])
            nc.gpsimd.dma_start(mk_sb[:], mk_ext[:])

            # ---- persistent activations ----
            qk_sb = acts.tile([P, 2, BT], BF16)          # [:,0,:]=q^T  [:,1,:]=k^T
            v_sb = acts.tile([P, 130 * NCH], BF16)
            nc.vector.memset(v_sb[:], 1.0)  # bakes in the ones columns

            # parity-doubled so back-to-back reps don't WAR-serialize on the
            # collective buffers
            a2a_in = [[dram.tile([NCORES, P, HTOK], BF16, name=f"a2ain{p}_{par}",
                                 tag=f"a2ain{p}_{par}") for p in range(NPH)]
                      for par in range(2)]
            a2a_out = [[dram.tile([NCORES, P, HTOK], BF16, name=f"a2aout{p}_{par}",
                                  tag=f"a2aout{p}_{par}") for p in range(NPH)]
                       for par in range(2)]

            me = nc.gpsimd if mask_pool else nc.vector

            def emit_qkv(tw):
                xw = xin.tile([P, 8, 512], BF16, tag="xw")
                nc.sync.dma_start(xw[:], xT_v[:, :, 512 * tw: 512 * (tw + 1)])
                pair = psum.tile([P, 2, 512], F32, tag="pair", bufs=pair_bufs)
                for cc in range(8):
                    nc.tensor.matmul(pair[:, 0, :], wq_sb[:, cc, :], xw[:, cc, :],
                                     start=(cc == 0), stop=(cc == 7))
                for cc in range(8):
                    nc.tensor.matmul(pair[:, 1, :], wk_sb[:, cc, :], xw[:, cc, :],
                                     start=(cc == 0), stop=(cc == 7))
                nc.vector.tensor_copy(qk_sb[:, :, 512 * tw: 512 * (tw + 1)], pair[:])
                # all four 128-token V chunks of this window share one PSUM
                # bank ([P, 4, 128] f32 = 2KB/partition)
                pv = psum.tile([P, 4, DH], F32, tag="pv", bufs=1)
                for c4 in range(4):
                    for cc in range(8):
                        nc.tensor.matmul(pv[:, c4, :], xw[:, cc, P * c4: P * (c4 + 1)],
                                         wv_sb[:, cc, :],
                                         start=(cc == 0), stop=(cc == 7))
                    gc = 4 * tw + c4
                    dst = v_sb[:, 130 * gc: 130 * gc + 130].rearrange(
                        "p (h d) -> p h d", h=2, d=65)[:, :, 0:64]
                    nc.vector.tensor_copy(dst, pv[:, c4, :].rearrange(
                        "p (h d) -> p h d", h=2))

            def emit_attn(b, qw, par=0):
                if "B" not in stages:
                    return
                q0 = Tb * b + 512 * qw
                kmax = 4 * qw + 4
                po = [psum.tile([P, 512], F32, tag="po", bufs=po_bufs,
                                name=f"po{lh}") for lh in range(HPC)]

                def s_chunk(kc):
                    k0 = Tb * b + P * kc
                    j = kc - 4 * qw
                    c0 = max(0, j) * P
                    pair = psum.tile([P, 2, 512], F32, tag="pair", bufs=pair_bufs)
                    for lh in range(HPC):
                        hs = 64 * lh
                        nc.tensor.matmul(
                            pair[:, lh, c0:512],
                            qk_sb[hs: hs + 64, 1, k0: k0 + P],
                            qk_sb[hs: hs + 64, 0, q0 + c0: q0 + 512],
                            start=True, stop=True,
                            tile_position=((hs, 0) if pack_s else None))
                    if bvariant == "sonly":
                        return (kc, c0, None)
                    pT = ptiles.tile([P, 2, 512], BF16, tag="pT")
                    func = (mybir.ActivationFunctionType.Copy
                            if bvariant == "noexp" else
                            mybir.ActivationFunctionType.Exp)
                    nc.scalar.activation(pT[:, :, c0:512], pair[:, :, c0:512],
                                         func, scale=float(SCALE))
                    if j >= 0:
                        # one multiply masks the 128-wide triangle block of
                        # both heads (columns past it are fully unmasked)
                        me.tensor_mul(pT[:, :, c0: c0 + P],
                                      pT[:, :, c0: c0 + P], mk_sb[:])
                    return (kc, c0, pT)

                def pv_chunk(st):
                    kc, c0, pT = st
                    if pT is None or bvariant == "nopv":
                        return
                    gc = (Tb // 128) * b + kc
                    for lh in range(HPC):
                        nc.tensor.matmul(
                            po[lh][0:65, c0:512],
                            v_sb[:, 130 * gc + 65 * lh: 130 * gc + 65 * lh + 65],
                            pT[:, lh, c0:512],
                            start=(kc == 0), stop=(kc == kmax - 1),
                            skip_group_check=True)

                sts = []
                for kc in range(kmax):
                    sts.append(s_chunk(kc))
                    if kc >= 1:
                        pv_chunk(sts[kc - 1])
                pv_chunk(sts[kmax - 1])
                if bvariant in ("sonly", "nopv", "nonorm"):
                    return
                r, ph = q0 // TOKS, (q0 % TOKS) // HTOK
                for lh in range(HPC):
                    hs = 64 * lh
                    rec = small.tile([1, 512], F32, tag="rec")
                    nc.vector.reciprocal(rec[:], po[lh][64:65, :])
                    rb = small.tile([64, 512], F32, tag="rb")
                    assert use_pbcast, "only the partition_broadcast path is implemented"
                    nc.gpsimd.partition_broadcast(rb[:], rec[:], channels=64)
                    ao = small.tile([64, 512], BF16, tag="ao")
                    nc.vector.tensor_mul(ao[:], po[lh][0:64, :], rb[:])
                    nc.sync.dma_start(a2a_in[par][ph][r, hs: hs + 64, :], ao[:])

            def fire_a2a(phase, par, eng):
                if skip_collective or "B" not in stages:
                    return
                eng.collective_compute(
                    "AllToAll",
                    mybir.AluOpType.bypass,
                    replica_groups=[list(range(NCORES))],
                    ins=[a2a_in[par][phase].opt()],
                    outs=[a2a_out[par][phase].opt()],
                )

            ga_tiles = {}

            def emit_ga(phase, par):
                if "C" not in stages:
                    return
                ga = acts.tile([P, 8, HTOK], BF16, name=f"ga{phase}",
                               tag=f"ga{phase}")
                nc.sync.dma_start(ga[:],
                                  a2a_out[par][phase].rearrange("j p t -> p j t"))
                ga_tiles[phase] = ga

            def emit_proj(phase, tc2_list):
                if "C" not in stages:
                    return
                ga = ga_tiles[phase]
                for tc2 in tc2_list:
                    for nw in range(NNW):
                        py = psum.tile([P, 512], F32, tag="po", bufs=po_bufs)
                        for cc in range(8):
                            nc.tensor.matmul(
                                py[:],
                                ga[:, cc, P * tc2: P * (tc2 + 1)],
                                wp_sb[:, cc, 512 * nw: 512 * (nw + 1)],
                                start=(cc == 0), stop=(cc == 7))
                        ys = small.tile([P, 512], F32, tag="ys")
                        nc.vector.tensor_copy(ys[:], py[:])
                        nc.sync.dma_start(
                            y_ext[HTOK * phase + P * tc2:
                                  HTOK * phase + P * (tc2 + 1),
                                  512 * nw: 512 * (nw + 1)],
                            ys[:])

            for rep in range(reps):
                par = rep % 2
                if Tb >= 2048:
                    # phase 0: qw 0 then qw 2, QKV interleaved
                    for b in range(B):
                        emit_qkv(4 * b)
                        emit_attn(b, 0, par)
                    for b in range(B):
                        emit_qkv(4 * b + 1)
                        emit_qkv(4 * b + 2)
                        emit_attn(b, 2, par)
                    # the collective blocks its issuing engine queue for the
                    # transfer duration, so it lives on Pool where only the
                    # (deferrable) masks/broadcasts queue behind it
                    fire_a2a(0, par, nc.gpsimd)
                    # phase 1 attention: qw 1 (no QKV needed), then qw 3,
                    # with phase-0 projection interleaved into the qw-3 groups.
                    # ga(0) is fetched only once the collective has had ~60us
                    # of attention work to complete, so its semaphore wait
                    # doesn't head-of-line-block the SP DMA queue.
                    for b in range(B):
                        emit_attn(b, 1, par)
                    proj0_chunks = {1: [0, 1], 2: [2], 3: [3]}
                    for b in range(B):
                        emit_qkv(4 * b + 3)
                        emit_attn(b, 3, par)
                        if b == 0:
                            emit_ga(0, par)
                        emit_proj(0, proj0_chunks.get(b, []))
                    # tail collective: Pool is idle by now
                    fire_a2a(1, par, nc.gpsimd)
                    emit_ga(1, par)
                    emit_proj(1, list(range(HTOK // P)))
                else:
                    for tw in range(NTW):
                        emit_qkv(tw)
                    for b in range(B):
                        for qw in range(NQW):
                            emit_attn(b, qw, par)
                    fire_a2a(0, par, nc.gpsimd)
                    emit_ga(0, par)
                    emit_proj(0, list(range(HTOK // P)))

    nc.finalize()
    return nc


def _host_inputs(x, w_attn, w_proj, Tb=T):
    import ml_dtypes
    bf16 = ml_dtypes.bfloat16
    BT = B * Tb
    xT = np.ascontiguousarray(x.reshape(BT, C).T).astype(bf16)
    wproj_bf = np.ascontiguousarray(w_proj).astype(bf16)
    rr = np.arange(P)[:, None]
    cc = np.arange(P)[None, :]
    tri = (rr <= cc).astype(bf16)            # keep key r <= query c
    masks = np.concatenate([tri, tri], axis=1)   # duplicated for both heads
    in_maps = []
    for g in range(NCORES):
        in_maps.append({
            "xT": xT,
            "wq": np.ascontiguousarray(w_attn[:, DH * g: DH * (g + 1)]).astype(bf16),
            "wk": np.ascontiguousarray(w_attn[:, C + DH * g: C + DH * (g + 1)]).astype(bf16),
            "wv": np.ascontiguousarray(w_attn[:, 2 * C + DH * g: 2 * C + DH * (g + 1)]).astype(bf16),
            "wproj": wproj_bf,
            "masks": masks,
        })
    return in_maps


_NC_CACHE = {}


def kernel(x, w_attn, w_proj):
    x = np.asarray(x)
    w_attn = np.asarray(w_attn)
    w_proj = np.asarray(w_proj)
    if T not in _NC_CACHE:
        _NC_CACHE[T] = build_nc(T)
    nc = _NC_CACHE[T]
    in_maps = _host_inputs(x, w_attn, w_proj, T)
    res = run_bass_kernel_spmd(nc, in_maps, core_ids=list(range(NCORES)))
    y = np.concatenate([res.results[g]["y"] for g in range(NCORES)], axis=0)
    return y.reshape(B, T, C).astype(np.float32)


# revision 26
# speedup vs baseline: 1.1383x; 1.0365x over previous
"""Distributed causal self-attention for 8 TRN2 NeuronCores.

Sharding: tensor-parallel over heads (2 heads/core, all batches), then an
on-device AllToAll redistributes the attention output from head-sharded to
token-sharded so each core computes a disjoint 1024-token slice of the
output projection.  Host work is only slicing / concatenation.

v3 pipeline design (HW-ablation driven):
  - QKV, attention and output projection are emitted in one interleaved
    stream ordered by token windows, so ACT(exp) starts almost immediately.
  - Q and K projections accumulate into one [128, 2, 512] PSUM pair tile
    (one DVE copy).  V goes through the v1 path: v^T matmul + PE transpose
    (few large matmuls; a direct [tok,dim] product costs 2x in unhidden
    LDWEIGHTS).
  - Both heads' S^T tiles live in one [128, 2, 512] PSUM pair -> ONE exp
    activation per chunk.
  - Causal masking costs no DVE/Pool work at all: one extra accumulating
    matmul (identity stationary x triangle constant) adds -400 to the
    masked triangle of the S pair before exp, which underflows to 0.
  - Softmax normalization: reciprocal + copy-out of the PSUM accumulator
    happen immediately (so the po bank frees fast); the broadcast matmul,
    final multiply and A2A-payload DMA are deferred by one attention group
    so their cross-engine latency never blocks the PE stream.
  - The Pool engine runs ONLY the collectives (they block their issuing
    engine's queue for the whole transfer, measured +99us when anything
    else queues behind them).  All DMA triggers stay on SP.
  - The phase-0 output projection is interleaved into the late attention
    groups; only the phase-1 A2A + projection is a tail.
"""

import numpy as np

import concourse.bass as bass
import concourse.bacc as bacc
import concourse.mybir as mybir
import concourse.tile as tile
from concourse.bass_utils import run_bass_kernel_spmd

B, T, C = 4, 2048, 1024
H, D = 16, 64
NCORES = 8
HPC = H // NCORES        # heads per core
DH = HPC * D             # 128 attention-output cols per core
P = 128
F32 = mybir.dt.float32
BF16 = mybir.dt.bfloat16
SCALE = 1.0 / np.sqrt(D)
MASK_NEG = -400.0        # exp(scale*(s-400)) == 0 in bf16 for any causal s


def build_nc(Tb=T, reps=1, stages="ABC", skip_collective=False, bvariant="full",
             pair_bufs=2, po_bufs=2, pt_bufs=8, vtp_bufs=1, tail_lag=True):
    """Build the SPMD Bass graph (identical on all 8 cores).

    reps > 1 emits the whole pipeline that many times (same buffers, so
    iterations serialize) -- used only for steady-state HW timing.
    """
    BT = B * Tb              # total tokens
    NQW = Tb // 512          # query windows per batch
    NCH = BT // 128          # 128-token chunks total
    TOKS = BT // NCORES      # tokens per core in the proj stage
    NNW = C // 512           # 512-wide output column windows
    NPH = 2 if Tb >= 2048 else 1
    HTOK = TOKS // NPH       # tokens per proj phase (A2A split)

    nc = bacc.Bacc(None, target_bir_lowering=False)

    xT_ext = nc.declare_dram_parameter("xT", [C, BT], BF16, isOutput=False)
    wq_ext = nc.declare_dram_parameter("wq", [C, DH], BF16, isOutput=False)
    wk_ext = nc.declare_dram_parameter("wk", [C, DH], BF16, isOutput=False)
    wv_ext = nc.declare_dram_parameter("wv", [C, DH], BF16, isOutput=False)
    wp_ext = nc.declare_dram_parameter("wproj", [C, C], BF16, isOutput=False)
    mk_ext = nc.declare_dram_parameter("masks", [P, 2 * P], BF16, isOutput=False)
    id_ext = nc.declare_dram_parameter("ident", [P, P], BF16, isOutput=False)
    y_ext = nc.declare_dram_parameter("y", [TOKS, C], F32, isOutput=True)

    xT_v = xT_ext.rearrange("(c p) t -> p c t", p=P)     # [128, 8, BT]
    wq_v = wq_ext.rearrange("(c p) m -> p c m", p=P)     # [128, 8, 128]
    wk_v = wk_ext.rearrange("(c p) m -> p c m", p=P)
    wv_v = wv_ext.rearrange("(c p) m -> p c m", p=P)
    wp_v = wp_ext.rearrange("(c p) m -> p c m", p=P)     # [128, 8, 1024]

    with tile.TileContext(nc, num_cores=NCORES) as tc:
        with (
            tc.tile_pool(name="consts", bufs=1) as consts,
            tc.tile_pool(name="acts", bufs=1) as acts,
            tc.tile_pool(name="xin", bufs=2) as xin,
            tc.tile_pool(name="small", bufs=4) as small,
            tc.tile_pool(name="ptiles", bufs=pt_bufs) as ptiles,
            tc.tile_pool(name="psum", bufs=1, space="PSUM") as psum,
            tc.tile_pool(name="dram", bufs=1, space="DRAM") as dram,
        ):
            # ---- constants ----
            wq_sb = consts.tile([P, 8, DH], BF16)
            wk_sb = consts.tile([P, 8, DH], BF16)
            wv_sb = consts.tile([P, 8, DH], BF16)
            wp_sb = consts.tile([P, 8, C], BF16)
            mk_sb = consts.tile([P, 2, P], BF16)   # -400 on masked triangle, x2 heads
            id_sb = consts.tile([P, P], BF16)
            ones_sb = consts.tile([1, P], BF16)
            nc.gpsimd.dma_start(wq_sb[:], wq_v[:])
            nc.gpsimd.dma_start(wk_sb[:], wk_v[:])
            nc.gpsimd.dma_start(wv_sb[:], wv_v[:])
            nc.gpsimd.dma_start(wp_sb[:], wp_v[:])
            nc.gpsimd.dma_start(mk_sb[:], mk_ext[:])
            nc.gpsimd.dma_start(id_sb[:], id_ext[:])
            nc.vector.memset(ones_sb[:], 1.0)

            # ---- persistent activations ----
            qk_sb = acts.tile([P, 2, BT], BF16)          # [:,0,:]=q^T  [:,1,:]=k^T
            v_sb = acts.tile([P, 130 * NCH], BF16)
            nc.vector.memset(v_sb[:], 1.0)  # bakes in the ones columns

            # parity-doubled so back-to-back reps don't WAR-serialize on the
            # collective buffers
            a2a_in = [[dram.tile([NCORES, P, HTOK], BF16, name=f"a2ain{p}_{par}",
                                 tag=f"a2ain{p}_{par}") for p in range(NPH)]
                      for par in range(2)]
            a2a_out = [[dram.tile([NCORES, P, HTOK], BF16, name=f"a2aout{p}_{par}",
                                  tag=f"a2aout{p}_{par}") for p in range(NPH)]
                       for par in range(2)]

            def emit_qkv(tw):
                xw = xin.tile([P, 8, 512], BF16, tag="xw")
                nc.sync.dma_start(xw[:], xT_v[:, :, 512 * tw: 512 * (tw + 1)])
                pair = psum.tile([P, 2, 512], F32, tag="pair", bufs=pair_bufs)
                for cc in range(8):
                    nc.tensor.matmul(pair[:, 0, :], wq_sb[:, cc, :], xw[:, cc, :],
                                     start=(cc == 0), stop=(cc == 7))
                for cc in range(8):
                    nc.tensor.matmul(pair[:, 1, :], wk_sb[:, cc, :], xw[:, cc, :],
                                     start=(cc == 0), stop=(cc == 7))
                nc.vector.tensor_copy(qk_sb[:, :, 512 * tw: 512 * (tw + 1)], pair[:])
                pairv = psum.tile([P, 2, 512], F32, tag="pair", bufs=pair_bufs)
                pvT = pairv[:, 0, :]
                for cc in range(8):
                    nc.tensor.matmul(pvT, wv_sb[:, cc, :], xw[:, cc, :],
                                     start=(cc == 0), stop=(cc == 7))
                vT_tmp = small.tile([P, 512], BF16, tag="vT")
                nc.vector.tensor_copy(vT_tmp[:], pvT)
                # all 4 transpose outputs share one half-bank PSUM tile;
                # per-slice dependency tracking keeps them pipelined
                vtp = psum.tile([P, 4, P], BF16, tag="vtp", bufs=vtp_bufs)
                for c4 in range(4):
                    nc.tensor.transpose(vtp[:, c4, :],
                                        vT_tmp[:, P * c4: P * (c4 + 1)],
                                        id_sb[:])
                    gc = 4 * tw + c4
                    dst = v_sb[:, 130 * gc: 130 * gc + 130].rearrange(
                        "p (h d) -> p h d", h=2, d=65)[:, :, 0:64]
                    nc.vector.tensor_copy(dst, vtp[:, c4, :].rearrange(
                        "p (h d) -> p h d", h=2))

            pending_tails = []

            def run_tails():
                while pending_tails:
                    pending_tails.pop(0)()

            def emit_attn(b, qw, par=0):
                if "B" not in stages:
                    return
                q0 = Tb * b + 512 * qw
                kmax = 4 * qw + 4
                po = [psum.tile([P, 512], F32, tag="po", bufs=po_bufs,
                                name=f"po{lh}") for lh in range(HPC)]

                def s_chunk(kc):
                    k0 = Tb * b + P * kc
                    j = kc - 4 * qw
                    c0 = max(0, j) * P
                    diag = j >= 0
                    pair = psum.tile([P, 2, 512], F32, tag="pair", bufs=pair_bufs)
                    for lh in range(HPC):
                        hs = 64 * lh
                        nc.tensor.matmul(
                            pair[:, lh, c0:512],
                            qk_sb[hs: hs + 64, 1, k0: k0 + P],
                            qk_sb[hs: hs + 64, 0, q0 + c0: q0 + 512],
                            start=True, stop=not diag,
                            skip_group_check=diag,
                            tile_position=(hs, 0))
                    if diag:
                        # accumulate -400 onto the causally masked triangle:
                        # identity (stationary) x triangle const, one matmul
                        # per head (a matmul can't span two PSUM banks)
                        for lh in range(HPC):
                            nc.tensor.matmul(
                                pair[:, lh, c0: c0 + P], id_sb[:],
                                mk_sb[:, lh, :],
                                start=False, stop=(lh == HPC - 1),
                                skip_group_check=True)
                    if bvariant == "sonly":
                        return (kc, c0, None)
                    pT = ptiles.tile([P, 2, 512], BF16, tag="pT")
                    func = (mybir.ActivationFunctionType.Copy
                            if bvariant == "noexp" else
                            mybir.ActivationFunctionType.Exp)
                    nc.scalar.activation(pT[:, :, c0:512], pair[:, :, c0:512],
                                         func, scale=float(SCALE))
                    return (kc, c0, pT)

                def pv_chunk(st):
                    kc, c0, pT = st
                    if pT is None or bvariant == "nopv":
                        return
                    gc = (Tb // 128) * b + kc
                    for lh in range(HPC):
                        nc.tensor.matmul(
                            po[lh][0:65, c0:512],
                            v_sb[:, 130 * gc + 65 * lh: 130 * gc + 65 * lh + 65],
                            pT[:, lh, c0:512],
                            start=(kc == 0), stop=(kc == kmax - 1),
                            skip_group_check=True)

                sts = []
                for kc in range(kmax):
                    sts.append(s_chunk(kc))
                    if kc >= 1:
                        pv_chunk(sts[kc - 1])
                pv_chunk(sts[kmax - 1])
                if bvariant in ("sonly", "nopv", "nonorm"):
                    return
                r, ph = q0 // TOKS, (q0 % TOKS) // HTOK
                # immediate part: reciprocal + copy-out, so the po banks free
                # after ~2 DVE ops instead of after the whole norm chain
                tails_in = []
                for lh in range(HPC):
                    rec = small.tile([1, 512], F32, tag="rec")
                    nc.vector.reciprocal(rec[:], po[lh][64:65, :])
                    rec_bf = small.tile([1, 512], BF16, tag="recb")
                    nc.vector.tensor_copy(rec_bf[:], rec[:])
                    o_sb = small.tile([64, 512], BF16, tag="osb", bufs=6)
                    nc.vector.tensor_copy(o_sb[:], po[lh][0:64, :])
                    tails_in.append((lh, rec_bf, o_sb))

                def tail():
                    for lh, rec_bf, o_sb in tails_in:
                        hs = 64 * lh
                        pb = psum.tile([P, 512], F32, tag="pb", bufs=1)
                        nc.tensor.matmul(pb[:], ones_sb[:], rec_bf[:],
                                         start=True, stop=True)
                        ao = small.tile([64, 512], BF16, tag="ao")
                        nc.vector.tensor_mul(ao[:], o_sb[:], pb[0:64, :])
                        nc.sync.dma_start(a2a_in[par][ph][r, hs: hs + 64, :],
                                          ao[:])

                if tail_lag:
                    pending_tails.append(tail)
                else:
                    tail()

            def fire_a2a(phase, par):
                run_tails()
                if skip_collective or "B" not in stages:
                    return
                # collectives block the issuing engine queue for the whole
                # transfer -> they get Pool, which runs nothing else
                nc.gpsimd.collective_compute(
                    "AllToAll",
                    mybir.AluOpType.bypass,
                    replica_groups=[list(range(NCORES))],
                    ins=[a2a_in[par][phase].opt()],
                    outs=[a2a_out[par][phase].opt()],
                )

            ga_tiles = {}

            def emit_ga(phase, par):
                if "C" not in stages:
                    return
                ga = acts.tile([P, 8, HTOK], BF16, name=f"ga{phase}",
                               tag=f"ga{phase}")
                nc.sync.dma_start(ga[:],
                                  a2a_out[par][phase].rearrange("j p t -> p j t"))
                ga_tiles[phase] = ga

            def emit_proj(phase, tc2_list):
                if "C" not in stages:
                    return
                ga = ga_tiles[phase]
                for tc2 in tc2_list:
                    for nw in range(NNW):
                        py = psum.tile([P, 512], F32, tag="po", bufs=po_bufs)
                        for cc in range(8):
                            nc.tensor.matmul(
                                py[:],
                                ga[:, cc, P * tc2: P * (tc2 + 1)],
                                wp_sb[:, cc, 512 * nw: 512 * (nw + 1)],
                                start=(cc == 0), stop=(cc == 7))
                        ys = small.tile([P, 512], F32, tag="ys")
                        nc.vector.tensor_copy(ys[:], py[:])
                        nc.sync.dma_start(
                            y_ext[HTOK * phase + P * tc2:
                                  HTOK * phase + P * (tc2 + 1),
                                  512 * nw: 512 * (nw + 1)],
                            ys[:])

            for rep in range(reps):
                par = rep % 2
                if Tb >= 2048:
                    # phase 0: qw 0 then qw 2, QKV interleaved
                    for b in range(B):
                        emit_qkv(4 * b)
                        emit_attn(b, 0, par)
                    for b in range(B):
                        emit_qkv(4 * b + 1)
                        emit_qkv(4 * b + 2)
                        emit_attn(b, 2, par)
                    fire_a2a(0, par)
                    # phase 1 attention: qw 1 (no QKV needed), then qw 3,
                    # with phase-0 projection interleaved into the qw-3
                    # groups.  ga(0) is fetched only after ~60us of attention
                    # has covered the collective, so its semaphore wait can't
                    # head-of-line-block the SP DMA queue.
                    for b in range(B):
                        emit_attn(b, 1, par)
                    proj0_chunks = {1: [0, 1], 2: [2], 3: [3]}
                    for b in range(B):
                        emit_qkv(4 * b + 3)
                        emit_attn(b, 3, par)
                        if b == 0:
                            emit_ga(0, par)
                        emit_proj(0, proj0_chunks.get(b, []))
                    fire_a2a(1, par)
                    emit_ga(1, par)
                    emit_proj(1, list(range(HTOK // P)))
                else:
                    for tw in range(B * NQW):
                        emit_qkv(tw)
                    for b in range(B):
                        for qw in range(NQW):
                            emit_attn(b, qw, par)
                    fire_a2a(0, par)
                    emit_ga(0, par)
                    emit_proj(0, list(range(HTOK // P)))

    nc.finalize()
    return nc


def _host_inputs(x, w_attn, w_proj, Tb=T):
    import ml_dtypes
    bf16 = ml_dtypes.bfloat16
    BT = B * Tb
    xT = np.ascontiguousarray(x.reshape(BT, C).T).astype(bf16)
    wproj_bf = np.ascontiguousarray(w_proj).astype(bf16)
    rr = np.arange(P)[:, None]
    cc = np.arange(P)[None, :]
    tri = np.where(rr > cc, np.float32(MASK_NEG), np.float32(0.0)).astype(bf16)
    masks = np.concatenate([tri, tri], axis=1)   # duplicated for both heads
    ident = np.eye(P).astype(bf16)
    in_maps = []
    for g in range(NCORES):
        in_maps.append({
            "xT": xT,
            "wq": np.ascontiguousarray(w_attn[:, DH * g: DH * (g + 1)]).astype(bf16),
            "wk": np.ascontiguousarray(w_attn[:, C + DH * g: C + DH * (g + 1)]).astype(bf16),
            "wv": np.ascontiguousarray(w_attn[:, 2 * C + DH * g: 2 * C + DH * (g + 1)]).astype(bf16),
            "wproj": wproj_bf,
            "masks": masks,
            "ident": ident,
        })
    return in_maps


_NC_CACHE = {}


def kernel(x, w_attn, w_proj):
    x = np.asarray(x)
    w_attn = np.asarray(w_attn)
    w_proj = np.asarray(w_proj)
    if T not in _NC_CACHE:
        _NC_CACHE[T] = build_nc(T)
    nc = _NC_CACHE[T]
    in_maps = _host_inputs(x, w_attn, w_proj, T)
    res = run_bass_kernel_spmd(nc, in_maps, core_ids=list(range(NCORES)))
    y = np.concatenate([res.results[g]["y"] for g in range(NCORES)], axis=0)
    return y.reshape(B, T, C).astype(np.float32)


# revision 33
# speedup vs baseline: 1.1811x; 1.0376x over previous
"""Distributed causal self-attention for 8 TRN2 NeuronCores.

Sharding: tensor-parallel over heads (2 heads/core, all batches), then an
on-device AllToAll redistributes the attention output from head-sharded to
token-sharded so each core computes a disjoint 1024-token slice of the
output projection.  Host work is only slicing / concatenation.

v3 pipeline design (HW-ablation driven):
  - QKV, attention and output projection are emitted in one interleaved
    stream ordered by token windows, so ACT(exp) starts almost immediately.
  - Q and K projections accumulate into one [128, 2, 512] PSUM pair tile
    (one DVE copy).  V goes through the v1 path: v^T matmul + PE transpose
    (few large matmuls; a direct [tok,dim] product costs 2x in unhidden
    LDWEIGHTS).
  - Both heads' S^T tiles live in one [128, 2, 512] PSUM pair -> ONE exp
    activation per chunk.
  - Causal masking costs no DVE/Pool work at all: one extra accumulating
    matmul (identity stationary x triangle constant) adds -400 to the
    masked triangle of the S pair before exp, which underflows to 0.
  - Softmax normalization: reciprocal + copy-out of the PSUM accumulator
    happen immediately (so the po bank frees fast); the broadcast matmul,
    final multiply and A2A-payload DMA are deferred by one attention group
    so their cross-engine latency never blocks the PE stream.
  - The Pool engine runs ONLY the collectives (they block their issuing
    engine's queue for the whole transfer, measured +99us when anything
    else queues behind them).  All DMA triggers stay on SP.
  - The phase-0 output projection is interleaved into the late attention
    groups; only the phase-1 A2A + projection is a tail.
"""

import numpy as np

import concourse.bass as bass
import concourse.bacc as bacc
import concourse.mybir as mybir
import concourse.tile as tile
from concourse import library_config
from concourse.bass_utils import run_bass_kernel_spmd

B, T, C = 4, 2048, 1024
H, D = 16, 64
NCORES = 8
HPC = H // NCORES        # heads per core
DH = HPC * D             # 128 attention-output cols per core
P = 128
F32 = mybir.dt.float32
BF16 = mybir.dt.bfloat16
SCALE = 1.0 / np.sqrt(D)
MASK_NEG = -400.0        # exp(scale*(s-400)) == 0 in bf16 for any causal s


def build_nc(Tb=T, reps=1, stages="ABC", skip_collective=False, bvariant="full",
             pair_bufs=2, po_bufs=3, pt_bufs=8, vtp_bufs=1, tail_lag=True):
    """Build the SPMD Bass graph (identical on all 8 cores).

    reps > 1 emits the whole pipeline that many times (same buffers, so
    iterations serialize) -- used only for steady-state HW timing.
    """
    BT = B * Tb              # total tokens
    NQW = Tb // 512          # query windows per batch
    NCH = BT // 128          # 128-token chunks total
    TOKS = BT // NCORES      # tokens per core in the proj stage
    NNW = C // 512           # 512-wide output column windows
    NPH = 2 if Tb >= 2048 else 1
    HTOK = TOKS // NPH       # tokens per proj phase (A2A split)

    nc = bacc.Bacc(None, target_bir_lowering=False)

    xT_ext = nc.declare_dram_parameter("xT", [C, BT], BF16, isOutput=False)
    wq_ext = nc.declare_dram_parameter("wq", [C, DH], BF16, isOutput=False)
    wk_ext = nc.declare_dram_parameter("wk", [C, DH], BF16, isOutput=False)
    wv_ext = nc.declare_dram_parameter("wv", [C, DH], BF16, isOutput=False)
    wp_ext = nc.declare_dram_parameter("wproj", [C, C], BF16, isOutput=False)
    mk_ext = nc.declare_dram_parameter("masks", [P, 2 * P], BF16, isOutput=False)
    id_ext = nc.declare_dram_parameter("ident", [P, P], BF16, isOutput=False)
    y_ext = nc.declare_dram_parameter("y", [TOKS, C], F32, isOutput=True)

    xT_v = xT_ext.rearrange("(c p) t -> p c t", p=P)     # [128, 8, BT]
    wq_v = wq_ext.rearrange("(c p) m -> p c m", p=P)     # [128, 8, 128]
    wk_v = wk_ext.rearrange("(c p) m -> p c m", p=P)
    wv_v = wv_ext.rearrange("(c p) m -> p c m", p=P)
    wp_v = wp_ext.rearrange("(c p) m -> p c m", p=P)     # [128, 8, 1024]

    with tile.TileContext(nc, num_cores=NCORES) as tc:
        with (
            tc.tile_pool(name="consts", bufs=1) as consts,
            tc.tile_pool(name="acts", bufs=1) as acts,
            tc.tile_pool(name="xin", bufs=2) as xin,
            tc.tile_pool(name="small", bufs=4) as small,
            tc.tile_pool(name="ptiles", bufs=pt_bufs) as ptiles,
            tc.tile_pool(name="psum", bufs=1, space="PSUM") as psum,
            tc.tile_pool(name="dram", bufs=1, space="DRAM") as dram,
        ):
            # GPSIMD ucode library with partition_broadcast (collectives are
            # not ucode and are unaffected)
            nc.gpsimd.load_library(library_config.attn)

            # ---- constants ----
            wq_sb = consts.tile([P, 8, DH], BF16)
            wk_sb = consts.tile([P, 8, DH], BF16)
            wv_sb = consts.tile([P, 8, DH], BF16)
            wp_sb = consts.tile([P, 8, C], BF16)
            mk_sb = consts.tile([P, 2, P], BF16)   # -400 on masked triangle, x2 heads
            id_sb = consts.tile([P, P], BF16)
            ones_sb = consts.tile([1, P], BF16)
            nc.gpsimd.dma_start(wq_sb[:], wq_v[:])
            nc.gpsimd.dma_start(wk_sb[:], wk_v[:])
            nc.gpsimd.dma_start(wv_sb[:], wv_v[:])
            nc.gpsimd.dma_start(wp_sb[:], wp_v[:])
            nc.gpsimd.dma_start(mk_sb[:], mk_ext[:])
            nc.gpsimd.dma_start(id_sb[:], id_ext[:])
            nc.vector.memset(ones_sb[:], 1.0)

            # ---- persistent activations ----
            qk_sb = acts.tile([P, 2, BT], BF16)          # [:,0,:]=q^T  [:,1,:]=k^T
            v_sb = acts.tile([P, 130 * NCH], BF16)
            nc.vector.memset(v_sb[:], 1.0)  # bakes in the ones columns

            # parity-doubled so back-to-back reps don't WAR-serialize on the
            # collective buffers
            a2a_in = [[dram.tile([NCORES, P, HTOK], BF16, name=f"a2ain{p}_{par}",
                                 tag=f"a2ain{p}_{par}") for p in range(NPH)]
                      for par in range(2)]
            a2a_out = [[dram.tile([NCORES, P, HTOK], BF16, name=f"a2aout{p}_{par}",
                                  tag=f"a2aout{p}_{par}") for p in range(NPH)]
                       for par in range(2)]

            def emit_qkv(tw):
                xw = xin.tile([P, 8, 512], BF16, tag="xw")
                nc.sync.dma_start(xw[:], xT_v[:, :, 512 * tw: 512 * (tw + 1)])
                pair = psum.tile([P, 2, 512], F32, tag="pair", bufs=pair_bufs)
                for cc in range(8):
                    nc.tensor.matmul(pair[:, 0, :], wq_sb[:, cc, :], xw[:, cc, :],
                                     start=(cc == 0), stop=(cc == 7))
                for cc in range(8):
                    nc.tensor.matmul(pair[:, 1, :], wk_sb[:, cc, :], xw[:, cc, :],
                                     start=(cc == 0), stop=(cc == 7))
                nc.vector.tensor_copy(qk_sb[:, :, 512 * tw: 512 * (tw + 1)], pair[:])
                pairv = psum.tile([P, 2, 512], F32, tag="pair", bufs=pair_bufs)
                pvT = pairv[:, 0, :]
                for cc in range(8):
                    nc.tensor.matmul(pvT, wv_sb[:, cc, :], xw[:, cc, :],
                                     start=(cc == 0), stop=(cc == 7))
                vT_tmp = small.tile([P, 512], BF16, tag="vT")
                nc.vector.tensor_copy(vT_tmp[:], pvT)
                # all 4 transpose outputs share one half-bank PSUM tile;
                # per-slice dependency tracking keeps them pipelined
                vtp = psum.tile([P, 4, P], BF16, tag="vtp", bufs=vtp_bufs)
                for c4 in range(4):
                    nc.tensor.transpose(vtp[:, c4, :],
                                        vT_tmp[:, P * c4: P * (c4 + 1)],
                                        id_sb[:])
                    gc = 4 * tw + c4
                    dst = v_sb[:, 130 * gc: 130 * gc + 130].rearrange(
                        "p (h d) -> p h d", h=2, d=65)[:, :, 0:64]
                    nc.vector.tensor_copy(dst, vtp[:, c4, :].rearrange(
                        "p (h d) -> p h d", h=2))

            pending_tails = []

            def run_tails():
                while pending_tails:
                    pending_tails.pop(0)()

            def emit_attn(b, qw, par=0):
                if "B" not in stages:
                    return
                q0 = Tb * b + 512 * qw
                kmax = 4 * qw + 4
                po = [psum.tile([P, 512], F32, tag="po", bufs=po_bufs,
                                name=f"po{lh}") for lh in range(HPC)]

                def s_chunk(kc):
                    k0 = Tb * b + P * kc
                    j = kc - 4 * qw
                    c0 = max(0, j) * P
                    diag = j >= 0
                    pair = psum.tile([P, 2, 512], F32, tag="pair", bufs=pair_bufs)
                    for lh in range(HPC):
                        hs = 64 * lh
                        nc.tensor.matmul(
                            pair[:, lh, c0:512],
                            qk_sb[hs: hs + 64, 1, k0: k0 + P],
                            qk_sb[hs: hs + 64, 0, q0 + c0: q0 + 512],
                            start=True, stop=not diag,
                            skip_group_check=diag,
                            tile_position=(hs, 0))
                    if diag:
                        # accumulate -400 onto the causally masked triangle:
                        # identity (stationary) x triangle const, one matmul
                        # per head (a matmul can't span two PSUM banks)
                        for lh in range(HPC):
                            nc.tensor.matmul(
                                pair[:, lh, c0: c0 + P], id_sb[:],
                                mk_sb[:, lh, :],
                                start=False, stop=(lh == HPC - 1),
                                skip_group_check=True)
                    if bvariant == "sonly":
                        return (kc, c0, None)
                    pT = ptiles.tile([P, 2, 512], BF16, tag="pT")
                    func = (mybir.ActivationFunctionType.Copy
                            if bvariant == "noexp" else
                            mybir.ActivationFunctionType.Exp)
                    nc.scalar.activation(pT[:, :, c0:512], pair[:, :, c0:512],
                                         func, scale=float(SCALE))
                    return (kc, c0, pT)

                def pv_chunk(st):
                    kc, c0, pT = st
                    if pT is None or bvariant == "nopv":
                        return
                    gc = (Tb // 128) * b + kc
                    for lh in range(HPC):
                        nc.tensor.matmul(
                            po[lh][0:65, c0:512],
                            v_sb[:, 130 * gc + 65 * lh: 130 * gc + 65 * lh + 65],
                            pT[:, lh, c0:512],
                            start=(kc == 0), stop=(kc == kmax - 1),
                            skip_group_check=True)

                sts = []
                for kc in range(kmax):
                    sts.append(s_chunk(kc))
                    if kc >= 1:
                        pv_chunk(sts[kc - 1])
                pv_chunk(sts[kmax - 1])
                if bvariant in ("sonly", "nopv", "nonorm"):
                    return
                r, ph = q0 // TOKS, (q0 % TOKS) // HTOK
                # immediate part: reciprocal + copy-out, so the po banks free
                # after ~2 DVE ops instead of after the whole norm chain
                tails_in = []
                for lh in range(HPC):
                    rec = small.tile([1, 512], F32, tag="rec")
                    nc.vector.reciprocal(rec[:], po[lh][64:65, :])
                    # broadcast on the (otherwise idle) GPSIMD engine right
                    # away; by the lagged tail it's long done
                    rb = small.tile([64, 512], F32, tag="rb", bufs=6)
                    nc.gpsimd.partition_broadcast(rb[:], rec[:], channels=64)
                    o_sb = small.tile([64, 512], BF16, tag="osb", bufs=6)
                    nc.vector.tensor_copy(o_sb[:], po[lh][0:64, :])
                    tails_in.append((lh, rb, o_sb))

                def tail():
                    for lh, rb, o_sb in tails_in:
                        hs = 64 * lh
                        ao = small.tile([64, 512], BF16, tag="ao")
                        nc.vector.tensor_mul(ao[:], o_sb[:], rb[:])
                        nc.sync.dma_start(a2a_in[par][ph][r, hs: hs + 64, :],
                                          ao[:])

                if tail_lag:
                    pending_tails.append(tail)
                else:
                    tail()

            def fire_a2a(phase, par):
                run_tails()
                if skip_collective or "B" not in stages:
                    return
                # collectives block the issuing engine queue for the whole
                # transfer -> they get Pool, which runs nothing else
                nc.gpsimd.collective_compute(
                    "AllToAll",
                    mybir.AluOpType.bypass,
                    replica_groups=[list(range(NCORES))],
                    ins=[a2a_in[par][phase].opt()],
                    outs=[a2a_out[par][phase].opt()],
                )

            ga_tiles = {}

            def emit_ga(phase, par):
                if "C" not in stages:
                    return
                ga = acts.tile([P, 8, HTOK], BF16, name=f"ga{phase}",
                               tag=f"ga{phase}")
                nc.sync.dma_start(ga[:],
                                  a2a_out[par][phase].rearrange("j p t -> p j t"))
                ga_tiles[phase] = ga

            def emit_proj(phase, tc2_list):
                if "C" not in stages:
                    return
                ga = ga_tiles[phase]
                for tc2 in tc2_list:
                    for nw in range(NNW):
                        py = psum.tile([P, 512], F32, tag="po", bufs=po_bufs)
                        for cc in range(8):
                            nc.tensor.matmul(
                                py[:],
                                ga[:, cc, P * tc2: P * (tc2 + 1)],
                                wp_sb[:, cc, 512 * nw: 512 * (nw + 1)],
                                start=(cc == 0), stop=(cc == 7))
                        ys = small.tile([P, 512], F32, tag="ys")
                        nc.vector.tensor_copy(ys[:], py[:])
                        nc.sync.dma_start(
                            y_ext[HTOK * phase + P * tc2:
                                  HTOK * phase + P * (tc2 + 1),
                                  512 * nw: 512 * (nw + 1)],
                            ys[:])

            for rep in range(reps):
                par = rep % 2
                if Tb >= 2048:
                    # phase 0: qw 0 then qw 2, QKV interleaved
                    for b in range(B):
                        emit_qkv(4 * b)
                        emit_attn(b, 0, par)
                    for b in range(B):
                        emit_qkv(4 * b + 1)
                        emit_qkv(4 * b + 2)
                        emit_attn(b, 2, par)
                    fire_a2a(0, par)
                    # phase 1 attention: qw 1 (no QKV needed), then qw 3,
                    # with phase-0 projection interleaved into the qw-3
                    # groups.  ga(0) is fetched only after ~60us of attention
                    # has covered the collective, so its semaphore wait can't
                    # head-of-line-block the SP DMA queue.
                    for b in range(B):
                        emit_attn(b, 1, par)
                    proj0_chunks = {1: [0, 1], 2: [2], 3: [3]}
                    for b in range(B):
                        emit_qkv(4 * b + 3)
                        emit_attn(b, 3, par)
                        if b == 0:
                            emit_ga(0, par)
                        emit_proj(0, proj0_chunks.get(b, []))
                    fire_a2a(1, par)
                    emit_ga(1, par)
                    emit_proj(1, list(range(HTOK // P)))
                else:
                    for tw in range(B * NQW):
                        emit_qkv(tw)
                    for b in range(B):
                        for qw in range(NQW):
                            emit_attn(b, qw, par)
                    fire_a2a(0, par)
                    emit_ga(0, par)
                    emit_proj(0, list(range(HTOK // P)))

    nc.finalize()
    return nc


def _host_inputs(x, w_attn, w_proj, Tb=T):
    import ml_dtypes
    bf16 = ml_dtypes.bfloat16
    BT = B * Tb
    xT = np.ascontiguousarray(x.reshape(BT, C).T).astype(bf16)
    wproj_bf = np.ascontiguousarray(w_proj).astype(bf16)
    rr = np.arange(P)[:, None]
    cc = np.arange(P)[None, :]
    tri = np.where(rr > cc, np.float32(MASK_NEG), np.float32(0.0)).astype(bf16)
    masks = np.concatenate([tri, tri], axis=1)   # duplicated for both heads
    ident = np.eye(P).astype(bf16)
    in_maps = []
    for g in range(NCORES):
        in_maps.append({
            "xT": xT,
            "wq": np.ascontiguousarray(w_attn[:, DH * g: DH * (g + 1)]).astype(bf16),
            "wk": np.ascontiguousarray(w_attn[:, C + DH * g: C + DH * (g + 1)]).astype(bf16),
            "wv": np.ascontiguousarray(w_attn[:, 2 * C + DH * g: 2 * C + DH * (g + 1)]).astype(bf16),
            "wproj": wproj_bf,
            "masks": masks,
            "ident": ident,
        })
    return in_maps


_NC_CACHE = {}


def kernel(x, w_attn, w_proj):
    x = np.asarray(x)
    w_attn = np.asarray(w_attn)
    w_proj = np.asarray(w_proj)
    if T not in _NC_CACHE:
        _NC_CACHE[T] = build_nc(T)
    nc = _NC_CACHE[T]
    in_maps = _host_inputs(x, w_attn, w_proj, T)
    res = run_bass_kernel_spmd(nc, in_maps, core_ids=list(range(NCORES)))
    y = np.concatenate([res.results[g]["y"] for g in range(NCORES)], axis=0)
    return y.reshape(B, T, C).astype(np.float32)


# revision 40
# speedup vs baseline: 1.2065x; 1.0215x over previous
"""Distributed causal self-attention for 8 TRN2 NeuronCores.

Sharding: tensor-parallel over heads (2 heads/core, all batches), then an
on-device AllToAll redistributes the attention output from head-sharded to
token-sharded so each core computes a disjoint 1024-token slice of the
output projection.  Host work is only slicing / concatenation.

v3 pipeline design (HW-ablation driven):
  - QKV, attention and output projection are emitted in one interleaved
    stream ordered by token windows, so ACT(exp) starts almost immediately.
  - Q and K projections accumulate into one [128, 2, 512] PSUM pair tile
    (one DVE copy).  V goes through the v1 path: v^T matmul + PE transpose
    (few large matmuls; a direct [tok,dim] product costs 2x in unhidden
    LDWEIGHTS).
  - Both heads' S^T tiles live in one [128, 2, 512] PSUM pair -> ONE exp
    activation per chunk.
  - Causal masking costs no DVE/Pool work at all: one extra accumulating
    matmul (identity stationary x triangle constant) adds -400 to the
    masked triangle of the S pair before exp, which underflows to 0.
  - Softmax normalization: reciprocal + copy-out of the PSUM accumulator
    happen immediately (so the po bank frees fast); the broadcast matmul,
    final multiply and A2A-payload DMA are deferred by one attention group
    so their cross-engine latency never blocks the PE stream.
  - The Pool engine runs ONLY the collectives (they block their issuing
    engine's queue for the whole transfer, measured +99us when anything
    else queues behind them).  All DMA triggers stay on SP.
  - The phase-0 output projection is interleaved into the late attention
    groups; only the phase-1 A2A + projection is a tail.
"""

import numpy as np

import concourse.bass as bass
import concourse.bacc as bacc
import concourse.mybir as mybir
import concourse.tile as tile
from concourse.bass_utils import run_bass_kernel_spmd

B, T, C = 4, 2048, 1024
H, D = 16, 64
NCORES = 8
HPC = H // NCORES        # heads per core
DH = HPC * D             # 128 attention-output cols per core
P = 128
F32 = mybir.dt.float32
BF16 = mybir.dt.bfloat16
SCALE = 1.0 / np.sqrt(D)
MASK_NEG = -400.0        # exp(scale*(s-400)) == 0 in bf16 for any causal s


def build_nc(Tb=T, reps=1, stages="ABC", skip_collective=False, bvariant="full",
             pair_bufs=2, po_bufs=2, pt_bufs=10, vtp_bufs=1, tail_lag=True):
    """Build the SPMD Bass graph (identical on all 8 cores).

    reps > 1 emits the whole pipeline that many times (same buffers, so
    iterations serialize) -- used only for steady-state HW timing.
    """
    BT = B * Tb              # total tokens
    NQW = Tb // 512          # query windows per batch
    NCH = BT // 128          # 128-token chunks total
    TOKS = BT // NCORES      # tokens per core in the proj stage
    NNW = C // 512           # 512-wide output column windows
    NPH = 2 if Tb >= 2048 else 1
    HTOK = TOKS // NPH       # tokens per proj phase (A2A split)

    nc = bacc.Bacc(None, target_bir_lowering=False)

    xT_ext = nc.declare_dram_parameter("xT", [C, BT], BF16, isOutput=False)
    wq_ext = nc.declare_dram_parameter("wq", [C, DH], BF16, isOutput=False)
    wk_ext = nc.declare_dram_parameter("wk", [C, DH], BF16, isOutput=False)
    wv_ext = nc.declare_dram_parameter("wv", [C, DH], BF16, isOutput=False)
    wp_ext = nc.declare_dram_parameter("wproj", [C, C], BF16, isOutput=False)
    mk_ext = nc.declare_dram_parameter("masks", [P, 2 * P], BF16, isOutput=False)
    id_ext = nc.declare_dram_parameter("ident", [P, P], BF16, isOutput=False)
    y_ext = nc.declare_dram_parameter("y", [TOKS, C], F32, isOutput=True)

    xT_v = xT_ext.rearrange("(c p) t -> p c t", p=P)     # [128, 8, BT]
    wq_v = wq_ext.rearrange("(c p) m -> p c m", p=P)     # [128, 8, 128]
    wk_v = wk_ext.rearrange("(c p) m -> p c m", p=P)
    wv_v = wv_ext.rearrange("(c p) m -> p c m", p=P)
    wp_v = wp_ext.rearrange("(c p) m -> p c m", p=P)     # [128, 8, 1024]

    with tile.TileContext(nc, num_cores=NCORES) as tc:
        with (
            tc.tile_pool(name="consts", bufs=1) as consts,
            tc.tile_pool(name="acts", bufs=1) as acts,
            tc.tile_pool(name="xin", bufs=2) as xin,
            tc.tile_pool(name="small", bufs=4) as small,
            tc.tile_pool(name="ptiles", bufs=pt_bufs) as ptiles,
            tc.tile_pool(name="psum", bufs=1, space="PSUM") as psum,
            tc.tile_pool(name="dram", bufs=1, space="DRAM") as dram,
        ):
            # ---- constants ----
            wq_sb = consts.tile([P, 8, DH], BF16)
            wk_sb = consts.tile([P, 8, DH], BF16)
            wv_sb = consts.tile([P, 8, DH], BF16)
            wp_sb = consts.tile([P, 8, C], BF16)
            mk_sb = consts.tile([P, 2, P], BF16)   # -400 on masked triangle, x2 heads
            id_sb = consts.tile([P, P], BF16)
            ones_sb = consts.tile([1, P], BF16)
            nc.gpsimd.dma_start(wq_sb[:], wq_v[:])
            nc.gpsimd.dma_start(wk_sb[:], wk_v[:])
            nc.gpsimd.dma_start(wv_sb[:], wv_v[:])
            nc.gpsimd.dma_start(wp_sb[:], wp_v[:])
            nc.gpsimd.dma_start(mk_sb[:], mk_ext[:])
            nc.gpsimd.dma_start(id_sb[:], id_ext[:])
            nc.vector.memset(ones_sb[:], 1.0)

            # ---- persistent activations ----
            qk_sb = acts.tile([P, 2, BT], BF16)          # [:,0,:]=q^T  [:,1,:]=k^T
            v_sb = acts.tile([P, 130 * NCH], BF16)
            nc.vector.memset(v_sb[:], 1.0)  # bakes in the ones columns

            # parity-doubled so back-to-back reps don't WAR-serialize on the
            # collective buffers
            a2a_in = [[dram.tile([NCORES, P, HTOK], BF16, name=f"a2ain{p}_{par}",
                                 tag=f"a2ain{p}_{par}") for p in range(NPH)]
                      for par in range(2)]
            a2a_out = [[dram.tile([NCORES, P, HTOK], BF16, name=f"a2aout{p}_{par}",
                                  tag=f"a2aout{p}_{par}") for p in range(NPH)]
                       for par in range(2)]

            def emit_qkv(tw):
                xw = xin.tile([P, 8, 512], BF16, tag="xw")
                nc.sync.dma_start(xw[:], xT_v[:, :, 512 * tw: 512 * (tw + 1)])
                pair = psum.tile([P, 2, 512], F32, tag="pair", bufs=pair_bufs)
                for cc in range(8):
                    nc.tensor.matmul(pair[:, 0, :], wq_sb[:, cc, :], xw[:, cc, :],
                                     start=(cc == 0), stop=(cc == 7))
                for cc in range(8):
                    nc.tensor.matmul(pair[:, 1, :], wk_sb[:, cc, :], xw[:, cc, :],
                                     start=(cc == 0), stop=(cc == 7))
                nc.vector.tensor_copy(qk_sb[:, :, 512 * tw: 512 * (tw + 1)], pair[:])
                pairv = psum.tile([P, 2, 512], F32, tag="pair", bufs=pair_bufs)
                pvT = pairv[:, 0, :]
                for cc in range(8):
                    nc.tensor.matmul(pvT, wv_sb[:, cc, :], xw[:, cc, :],
                                     start=(cc == 0), stop=(cc == 7))
                vT_tmp = small.tile([P, 512], BF16, tag="vT")
                nc.vector.tensor_copy(vT_tmp[:], pvT)
                # all 4 transpose outputs share one half-bank PSUM tile;
                # per-slice dependency tracking keeps them pipelined
                vtp = psum.tile([P, 4, P], BF16, tag="vtp", bufs=vtp_bufs)
                for c4 in range(4):
                    nc.tensor.transpose(vtp[:, c4, :],
                                        vT_tmp[:, P * c4: P * (c4 + 1)],
                                        id_sb[:])
                    gc = 4 * tw + c4
                    dst = v_sb[:, 130 * gc: 130 * gc + 130].rearrange(
                        "p (h d) -> p h d", h=2, d=65)[:, :, 0:64]
                    nc.vector.tensor_copy(dst, vtp[:, c4, :].rearrange(
                        "p (h d) -> p h d", h=2))

            pending_tails = []
            TAIL_LAG = 2   # groups of cover before a norm tail executes

            def push_tail(t):
                pending_tails.append(t)
                while len(pending_tails) > TAIL_LAG:
                    pending_tails.pop(0)()

            def run_tails():
                while pending_tails:
                    pending_tails.pop(0)()

            def emit_attn(b, qw, par=0):
                if "B" not in stages:
                    return
                q0 = Tb * b + 512 * qw
                kmax = 4 * qw + 4
                po = [psum.tile([P, 512], F32, tag="po", bufs=po_bufs,
                                name=f"po{lh}") for lh in range(HPC)]

                def s_chunk(kc):
                    k0 = Tb * b + P * kc
                    j = kc - 4 * qw
                    c0 = max(0, j) * P
                    diag = j >= 0
                    pair = psum.tile([P, 2, 512], F32, tag="pair", bufs=pair_bufs)
                    for lh in range(HPC):
                        hs = 64 * lh
                        nc.tensor.matmul(
                            pair[:, lh, c0:512],
                            qk_sb[hs: hs + 64, 1, k0: k0 + P],
                            qk_sb[hs: hs + 64, 0, q0 + c0: q0 + 512],
                            start=True, stop=not diag,
                            skip_group_check=diag,
                            tile_position=(hs, 0))
                    if diag:
                        # accumulate -400 onto the causally masked triangle:
                        # identity (stationary) x triangle const, one matmul
                        # per head (a matmul can't span two PSUM banks)
                        for lh in range(HPC):
                            nc.tensor.matmul(
                                pair[:, lh, c0: c0 + P], id_sb[:],
                                mk_sb[:, lh, :],
                                start=False, stop=(lh == HPC - 1),
                                skip_group_check=True)
                    if bvariant == "sonly":
                        return (kc, c0, None)
                    pT = ptiles.tile([P, 2, 512], BF16, tag="pT")
                    func = (mybir.ActivationFunctionType.Copy
                            if bvariant == "noexp" else
                            mybir.ActivationFunctionType.Exp)
                    nc.scalar.activation(pT[:, :, c0:512], pair[:, :, c0:512],
                                         func, scale=float(SCALE))
                    return (kc, c0, pT)

                def pv_chunk(st):
                    kc, c0, pT = st
                    if pT is None or bvariant == "nopv":
                        return
                    gc = (Tb // 128) * b + kc
                    for lh in range(HPC):
                        nc.tensor.matmul(
                            po[lh][0:65, c0:512],
                            v_sb[:, 130 * gc + 65 * lh: 130 * gc + 65 * lh + 65],
                            pT[:, lh, c0:512],
                            start=(kc == 0), stop=(kc == kmax - 1),
                            skip_group_check=True)

                sts = []
                for kc in range(kmax):
                    sts.append(s_chunk(kc))
                    if kc >= 1:
                        pv_chunk(sts[kc - 1])
                pv_chunk(sts[kmax - 1])
                if bvariant in ("sonly", "nopv", "nonorm"):
                    return
                r, ph = q0 // TOKS, (q0 % TOKS) // HTOK
                # immediate part: reciprocal + copy-out, so the po banks free
                # after ~2 DVE ops instead of after the whole norm chain
                tails_in = []
                for lh in range(HPC):
                    rec = small.tile([1, 512], F32, tag="rec")
                    nc.vector.reciprocal(rec[:], po[lh][64:65, :])
                    rec_bf = small.tile([1, 512], BF16, tag="recb", bufs=6)
                    nc.vector.tensor_copy(rec_bf[:], rec[:])
                    o_sb = small.tile([64, 512], BF16, tag="osb", bufs=6)
                    nc.vector.tensor_copy(o_sb[:], po[lh][0:64, :])
                    tails_in.append((lh, rec_bf, o_sb))

                def tail():
                    for lh, rec_bf, o_sb in tails_in:
                        hs = 64 * lh
                        # broadcast 1/denominator across partitions with a
                        # rank-1 PE matmul; two groups of lag hide its inputs
                        pb = psum.tile([P, 512], F32, tag="pb", bufs=1)
                        nc.tensor.matmul(pb[:], ones_sb[:], rec_bf[:],
                                         start=True, stop=True)
                        ao = small.tile([64, 512], BF16, tag="ao")
                        nc.vector.tensor_mul(ao[:], o_sb[:], pb[0:64, :])
                        nc.sync.dma_start(a2a_in[par][ph][r, hs: hs + 64, :],
                                          ao[:])

                if tail_lag:
                    push_tail(tail)
                else:
                    tail()

            def fire_a2a(phase, par):
                run_tails()
                if skip_collective or "B" not in stages:
                    return
                # collectives block the issuing engine queue for the whole
                # transfer -> they get Pool, which runs nothing else
                nc.gpsimd.collective_compute(
                    "AllToAll",
                    mybir.AluOpType.bypass,
                    replica_groups=[list(range(NCORES))],
                    ins=[a2a_in[par][phase].opt()],
                    outs=[a2a_out[par][phase].opt()],
                )

            ga_tiles = {}

            def emit_ga(phase, par):
                if "C" not in stages:
                    return
                ga = acts.tile([P, 8, HTOK], BF16, name=f"ga{phase}",
                               tag=f"ga{phase}")
                nc.sync.dma_start(ga[:],
                                  a2a_out[par][phase].rearrange("j p t -> p j t"))
                ga_tiles[phase] = ga

            def emit_proj(phase, tc2_list):
                if "C" not in stages:
                    return
                ga = ga_tiles[phase]
                for tc2 in tc2_list:
                    for nw in range(NNW):
                        py = psum.tile([P, 512], F32, tag="po", bufs=po_bufs)
                        for cc in range(8):
                            nc.tensor.matmul(
                                py[:],
                                ga[:, cc, P * tc2: P * (tc2 + 1)],
                                wp_sb[:, cc, 512 * nw: 512 * (nw + 1)],
                                start=(cc == 0), stop=(cc == 7))
                        ys = small.tile([P, 512], F32, tag="ys")
                        nc.vector.tensor_copy(ys[:], py[:])
                        nc.sync.dma_start(
                            y_ext[HTOK * phase + P * tc2:
                                  HTOK * phase + P * (tc2 + 1),
                                  512 * nw: 512 * (nw + 1)],
                            ys[:])

            for rep in range(reps):
                par = rep % 2
                if Tb >= 2048:
                    # phase 0: qw 0 then qw 2, QKV interleaved
                    for b in range(B):
                        emit_qkv(4 * b)
                        emit_attn(b, 0, par)
                    for b in range(B):
                        emit_qkv(4 * b + 1)
                        emit_qkv(4 * b + 2)
                        emit_attn(b, 2, par)
                    fire_a2a(0, par)
                    # phase 1 attention: qw 1 (no QKV needed), then qw 3,
                    # with phase-0 projection interleaved into the qw-3
                    # groups.  ga(0) is fetched only after ~60us of attention
                    # has covered the collective, so its semaphore wait can't
                    # head-of-line-block the SP DMA queue.
                    for b in range(B):
                        emit_attn(b, 1, par)
                    proj0_chunks = {1: [0, 1], 2: [2], 3: [3]}
                    for b in range(B):
                        emit_qkv(4 * b + 3)
                        emit_attn(b, 3, par)
                        if b == 0:
                            emit_ga(0, par)
                        emit_proj(0, proj0_chunks.get(b, []))
                    fire_a2a(1, par)
                    emit_ga(1, par)
                    emit_proj(1, list(range(HTOK // P)))
                else:
                    for tw in range(B * NQW):
                        emit_qkv(tw)
                    for b in range(B):
                        for qw in range(NQW):
                            emit_attn(b, qw, par)
                    fire_a2a(0, par)
                    emit_ga(0, par)
                    emit_proj(0, list(range(HTOK // P)))

    nc.finalize()
    return nc


def _host_inputs(x, w_attn, w_proj, Tb=T):
    import ml_dtypes
    bf16 = ml_dtypes.bfloat16
    BT = B * Tb
    xT = np.ascontiguousarray(x.reshape(BT, C).T).astype(bf16)
    wproj_bf = np.ascontiguousarray(w_proj).astype(bf16)
    rr = np.arange(P)[:, None]
    cc = np.arange(P)[None, :]
    tri = np.where(rr > cc, np.float32(MASK_NEG), np.float32(0.0)).astype(bf16)
    masks = np.concatenate([tri, tri], axis=1)   # duplicated for both heads
    ident = np.eye(P).astype(bf16)
    in_maps = []
    for g in range(NCORES):
        in_maps.append({
            "xT": xT,
            "wq": np.ascontiguousarray(w_attn[:, DH * g: DH * (g + 1)]).astype(bf16),
            "wk": np.ascontiguousarray(w_attn[:, C + DH * g: C + DH * (g + 1)]).astype(bf16),
            "wv": np.ascontiguousarray(w_attn[:, 2 * C + DH * g: 2 * C + DH * (g + 1)]).astype(bf16),
            "wproj": wproj_bf,
            "masks": masks,
            "ident": ident,
        })
    return in_maps


_NC_CACHE = {}


def kernel(x, w_attn, w_proj):
    x = np.asarray(x)
    w_attn = np.asarray(w_attn)
    w_proj = np.asarray(w_proj)
    if T not in _NC_CACHE:
        _NC_CACHE[T] = build_nc(T)
    nc = _NC_CACHE[T]
    in_maps = _host_inputs(x, w_attn, w_proj, T)
    res = run_bass_kernel_spmd(nc, in_maps, core_ids=list(range(NCORES)))
    y = np.concatenate([res.results[g]["y"] for g in range(NCORES)], axis=0)
    return y.reshape(B, T, C).astype(np.float32)


# revision 43
# speedup vs baseline: 1.2237x; 1.0142x over previous
"""Distributed causal self-attention for 8 TRN2 NeuronCores.

Sharding: tensor-parallel over heads (2 heads/core, all batches), then an
on-device AllToAll redistributes the attention output from head-sharded to
token-sharded so each core computes a disjoint 1024-token slice of the
output projection.  Host work is only slicing / concatenation.

v3 pipeline design (HW-ablation driven):
  - QKV, attention and output projection are emitted in one interleaved
    stream ordered by token windows, so ACT(exp) starts almost immediately.
  - Q and K projections accumulate into one [128, 2, 512] PSUM pair tile
    (one DVE copy).  V goes through the v1 path: v^T matmul + PE transpose
    (few large matmuls; a direct [tok,dim] product costs 2x in unhidden
    LDWEIGHTS).
  - Both heads' S^T tiles live in one [128, 2, 512] PSUM pair -> ONE exp
    activation per chunk.
  - Causal masking costs no DVE/Pool work at all: one extra accumulating
    matmul (identity stationary x triangle constant) adds -400 to the
    masked triangle of the S pair before exp, which underflows to 0.
  - Softmax normalization: reciprocal + copy-out of the PSUM accumulator
    happen immediately (so the po bank frees fast); the broadcast matmul,
    final multiply and A2A-payload DMA are deferred by one attention group
    so their cross-engine latency never blocks the PE stream.
  - The Pool engine runs ONLY the collectives (they block their issuing
    engine's queue for the whole transfer, measured +99us when anything
    else queues behind them).  All DMA triggers stay on SP.
  - The phase-0 output projection is interleaved into the late attention
    groups; only the phase-1 A2A + projection is a tail.
"""

import numpy as np

import concourse.bass as bass
import concourse.bacc as bacc
import concourse.mybir as mybir
import concourse.tile as tile
from concourse.bass_utils import run_bass_kernel_spmd

B, T, C = 4, 2048, 1024
H, D = 16, 64
NCORES = 8
HPC = H // NCORES        # heads per core
DH = HPC * D             # 128 attention-output cols per core
P = 128
F32 = mybir.dt.float32
BF16 = mybir.dt.bfloat16
SCALE = 1.0 / np.sqrt(D)
MASK_NEG = -400.0        # exp(scale*(s-400)) == 0 in bf16 for any causal s


def build_nc(Tb=T, reps=1, stages="ABC", skip_collective=False, bvariant="full",
             pair_bufs=2, po_bufs=2, pt_bufs=10, vtp_bufs=1, tail_lag=True):
    """Build the SPMD Bass graph (identical on all 8 cores).

    reps > 1 emits the whole pipeline that many times (same buffers, so
    iterations serialize) -- used only for steady-state HW timing.
    """
    BT = B * Tb              # total tokens
    NQW = Tb // 512          # query windows per batch
    NCH = BT // 128          # 128-token chunks total
    TOKS = BT // NCORES      # tokens per core in the proj stage
    NNW = C // 512           # 512-wide output column windows
    NPH = 2 if Tb >= 2048 else 1
    HTOK = TOKS // NPH       # tokens per proj phase (A2A split)

    nc = bacc.Bacc(None, target_bir_lowering=False)

    xT_ext = nc.declare_dram_parameter("xT", [C, BT], BF16, isOutput=False)
    wq_ext = nc.declare_dram_parameter("wq", [C, DH], BF16, isOutput=False)
    wk_ext = nc.declare_dram_parameter("wk", [C, DH], BF16, isOutput=False)
    wv_ext = nc.declare_dram_parameter("wv", [C, DH], BF16, isOutput=False)
    wp_ext = nc.declare_dram_parameter("wproj", [C, C], BF16, isOutput=False)
    mk_ext = nc.declare_dram_parameter("masks", [P, 2 * P], BF16, isOutput=False)
    id_ext = nc.declare_dram_parameter("ident", [P, P], BF16, isOutput=False)
    y_ext = nc.declare_dram_parameter("y", [TOKS, C], F32, isOutput=True)

    xT_v = xT_ext.rearrange("(c p) t -> p c t", p=P)     # [128, 8, BT]
    wq_v = wq_ext.rearrange("(c p) m -> p c m", p=P)     # [128, 8, 128]
    wk_v = wk_ext.rearrange("(c p) m -> p c m", p=P)
    wv_v = wv_ext.rearrange("(c p) m -> p c m", p=P)
    wp_v = wp_ext.rearrange("(c p) m -> p c m", p=P)     # [128, 8, 1024]

    with tile.TileContext(nc, num_cores=NCORES) as tc:
        with (
            tc.tile_pool(name="consts", bufs=1) as consts,
            tc.tile_pool(name="acts", bufs=1) as acts,
            tc.tile_pool(name="xin", bufs=2) as xin,
            tc.tile_pool(name="small", bufs=4) as small,
            tc.tile_pool(name="ptiles", bufs=pt_bufs) as ptiles,
            tc.tile_pool(name="psum", bufs=1, space="PSUM") as psum,
            tc.tile_pool(name="dram", bufs=1, space="DRAM") as dram,
        ):
            # ---- constants ----
            wq_sb = consts.tile([P, 8, DH], BF16)
            wk_sb = consts.tile([P, 8, DH], BF16)
            wv_sb = consts.tile([P, 8, DH], BF16)
            wp_sb = consts.tile([P, 8, C], BF16)
            mk_sb = consts.tile([P, 2, P], BF16)   # -400 on masked triangle, x2 heads
            id_sb = consts.tile([P, P], BF16)
            ones_sb = consts.tile([1, P], BF16)
            nc.gpsimd.dma_start(wq_sb[:], wq_v[:])
            nc.gpsimd.dma_start(wk_sb[:], wk_v[:])
            nc.gpsimd.dma_start(wv_sb[:], wv_v[:])
            nc.gpsimd.dma_start(wp_sb[:], wp_v[:])
            nc.gpsimd.dma_start(mk_sb[:], mk_ext[:])
            nc.gpsimd.dma_start(id_sb[:], id_ext[:])
            nc.vector.memset(ones_sb[:], 1.0)

            # ---- persistent activations ----
            qk_sb = acts.tile([P, 2, BT], BF16)          # [:,0,:]=q^T  [:,1,:]=k^T
            v_sb = acts.tile([P, 130 * NCH], BF16)
            nc.vector.memset(v_sb[:], 1.0)  # bakes in the ones columns

            # parity-doubled so back-to-back reps don't WAR-serialize on the
            # collective buffers
            a2a_in = [[dram.tile([NCORES, P, HTOK], BF16, name=f"a2ain{p}_{par}",
                                 tag=f"a2ain{p}_{par}") for p in range(NPH)]
                      for par in range(2)]
            a2a_out = [[dram.tile([NCORES, P, HTOK], BF16, name=f"a2aout{p}_{par}",
                                  tag=f"a2aout{p}_{par}") for p in range(NPH)]
                       for par in range(2)]

            def emit_qkv(tw):
                xw = xin.tile([P, 8, 512], BF16, tag="xw")
                nc.sync.dma_start(xw[:], xT_v[:, :, 512 * tw: 512 * (tw + 1)])
                pair = psum.tile([P, 2, 512], F32, tag="pair", bufs=pair_bufs)
                for cc in range(8):
                    nc.tensor.matmul(pair[:, 0, :], wq_sb[:, cc, :], xw[:, cc, :],
                                     start=(cc == 0), stop=(cc == 7))
                for cc in range(8):
                    nc.tensor.matmul(pair[:, 1, :], wk_sb[:, cc, :], xw[:, cc, :],
                                     start=(cc == 0), stop=(cc == 7))
                nc.vector.tensor_copy(qk_sb[:, :, 512 * tw: 512 * (tw + 1)], pair[:])
                pairv = psum.tile([P, 2, 512], F32, tag="pair", bufs=pair_bufs)
                pvT = pairv[:, 0, :]
                for cc in range(8):
                    nc.tensor.matmul(pvT, wv_sb[:, cc, :], xw[:, cc, :],
                                     start=(cc == 0), stop=(cc == 7))
                vT_tmp = small.tile([P, 512], BF16, tag="vT")
                nc.vector.tensor_copy(vT_tmp[:], pvT)
                # all 4 transpose outputs share one half-bank PSUM tile;
                # per-slice dependency tracking keeps them pipelined
                vtp = psum.tile([P, 4, P], BF16, tag="vtp", bufs=vtp_bufs)
                for c4 in range(4):
                    nc.tensor.transpose(vtp[:, c4, :],
                                        vT_tmp[:, P * c4: P * (c4 + 1)],
                                        id_sb[:])
                    gc = 4 * tw + c4
                    dst = v_sb[:, 130 * gc: 130 * gc + 130].rearrange(
                        "p (h d) -> p h d", h=2, d=65)[:, :, 0:64]
                    nc.vector.tensor_copy(dst, vtp[:, c4, :].rearrange(
                        "p (h d) -> p h d", h=2))

            pending_tails = []
            TAIL_LAG = 4   # per-head tails of cover (~2 attention groups)

            def push_tail(t):
                pending_tails.append(t)
                while len(pending_tails) > TAIL_LAG:
                    pending_tails.pop(0)()

            def drain_one():
                # called mid-group so consecutive per-head tails are separated
                # by S matmuls (the broadcast matmul then never waits on the
                # previous tail's DVE multiply)
                if len(pending_tails) > TAIL_LAG - 1:
                    pending_tails.pop(0)()

            def run_tails():
                while pending_tails:
                    pending_tails.pop(0)()

            def emit_attn(b, qw, par=0):
                if "B" not in stages:
                    return
                q0 = Tb * b + 512 * qw
                kmax = 4 * qw + 4
                po = [psum.tile([P, 512], F32, tag="po", bufs=po_bufs,
                                name=f"po{lh}") for lh in range(HPC)]

                def s_chunk(kc):
                    k0 = Tb * b + P * kc
                    j = kc - 4 * qw
                    c0 = max(0, j) * P
                    diag = j >= 0
                    pair = psum.tile([P, 2, 512], F32, tag="pair", bufs=pair_bufs)
                    for lh in range(HPC):
                        hs = 64 * lh
                        nc.tensor.matmul(
                            pair[:, lh, c0:512],
                            qk_sb[hs: hs + 64, 1, k0: k0 + P],
                            qk_sb[hs: hs + 64, 0, q0 + c0: q0 + 512],
                            start=True, stop=not diag,
                            skip_group_check=diag,
                            tile_position=(hs, 0))
                    if diag:
                        # accumulate -400 onto the causally masked triangle:
                        # identity (stationary) x triangle const, one matmul
                        # per head (a matmul can't span two PSUM banks)
                        for lh in range(HPC):
                            nc.tensor.matmul(
                                pair[:, lh, c0: c0 + P], id_sb[:],
                                mk_sb[:, lh, :],
                                start=False, stop=(lh == HPC - 1),
                                skip_group_check=True)
                    if bvariant == "sonly":
                        return (kc, c0, None)
                    pT = ptiles.tile([P, 2, 512], BF16, tag="pT")
                    func = (mybir.ActivationFunctionType.Copy
                            if bvariant == "noexp" else
                            mybir.ActivationFunctionType.Exp)
                    nc.scalar.activation(pT[:, :, c0:512], pair[:, :, c0:512],
                                         func, scale=float(SCALE))
                    return (kc, c0, pT)

                def pv_chunk(st):
                    kc, c0, pT = st
                    if pT is None or bvariant == "nopv":
                        return
                    gc = (Tb // 128) * b + kc
                    for lh in range(HPC):
                        nc.tensor.matmul(
                            po[lh][0:65, c0:512],
                            v_sb[:, 130 * gc + 65 * lh: 130 * gc + 65 * lh + 65],
                            pT[:, lh, c0:512],
                            start=(kc == 0), stop=(kc == kmax - 1),
                            skip_group_check=True)

                sts = []
                for kc in range(kmax):
                    sts.append(s_chunk(kc))
                    if kc == 1 and tail_lag:
                        drain_one()
                    if kc >= 1:
                        pv_chunk(sts[kc - 1])
                pv_chunk(sts[kmax - 1])
                if bvariant in ("sonly", "nopv", "nonorm"):
                    return
                r, ph = q0 // TOKS, (q0 % TOKS) // HTOK
                # immediate part: reciprocal + copy-out, so the po banks free
                # after ~2 DVE ops instead of after the whole norm chain
                tails_in = []
                for lh in range(HPC):
                    rec = small.tile([1, 512], F32, tag="rec")
                    nc.vector.reciprocal(rec[:], po[lh][64:65, :])
                    rec_bf = small.tile([1, 512], BF16, tag="recb", bufs=6)
                    nc.vector.tensor_copy(rec_bf[:], rec[:])
                    o_sb = small.tile([64, 512], BF16, tag="osb", bufs=6)
                    nc.vector.tensor_copy(o_sb[:], po[lh][0:64, :])
                    tails_in.append((lh, rec_bf, o_sb))

                def make_tail(lh, rec_bf, o_sb):
                    def tail():
                        hs = 64 * lh
                        # broadcast 1/denominator across partitions with a
                        # rank-1 PE matmul; ~2 groups of lag hide its inputs
                        pb = psum.tile([P, 512], F32, tag="pb", bufs=1)
                        nc.tensor.matmul(pb[:], ones_sb[:], rec_bf[:],
                                         start=True, stop=True)
                        ao = small.tile([64, 512], BF16, tag="ao")
                        nc.vector.tensor_mul(ao[:], o_sb[:], pb[0:64, :])
                        nc.sync.dma_start(a2a_in[par][ph][r, hs: hs + 64, :],
                                          ao[:])
                    return tail

                for lh, rec_bf, o_sb in tails_in:
                    t = make_tail(lh, rec_bf, o_sb)
                    if tail_lag:
                        push_tail(t)
                    else:
                        t()

            def fire_a2a(phase, par):
                run_tails()
                if skip_collective or "B" not in stages:
                    return
                # collectives block the issuing engine queue for the whole
                # transfer -> they get Pool, which runs nothing else
                nc.gpsimd.collective_compute(
                    "AllToAll",
                    mybir.AluOpType.bypass,
                    replica_groups=[list(range(NCORES))],
                    ins=[a2a_in[par][phase].opt()],
                    outs=[a2a_out[par][phase].opt()],
                )

            ga_tiles = {}

            def emit_ga(phase, par):
                if "C" not in stages:
                    return
                ga = acts.tile([P, 8, HTOK], BF16, name=f"ga{phase}",
                               tag=f"ga{phase}")
                nc.sync.dma_start(ga[:],
                                  a2a_out[par][phase].rearrange("j p t -> p j t"))
                ga_tiles[phase] = ga

            def emit_proj(phase, tc2_list):
                if "C" not in stages:
                    return
                ga = ga_tiles[phase]
                for tc2 in tc2_list:
                    for nw in range(NNW):
                        py = psum.tile([P, 512], F32, tag="po", bufs=po_bufs)
                        for cc in range(8):
                            nc.tensor.matmul(
                                py[:],
                                ga[:, cc, P * tc2: P * (tc2 + 1)],
                                wp_sb[:, cc, 512 * nw: 512 * (nw + 1)],
                                start=(cc == 0), stop=(cc == 7))
                        ys = small.tile([P, 512], F32, tag="ys")
                        nc.vector.tensor_copy(ys[:], py[:])
                        nc.sync.dma_start(
                            y_ext[HTOK * phase + P * tc2:
                                  HTOK * phase + P * (tc2 + 1),
                                  512 * nw: 512 * (nw + 1)],
                            ys[:])

            for rep in range(reps):
                par = rep % 2
                if Tb >= 2048:
                    # phase 0: qw 0 then qw 2, QKV interleaved
                    for b in range(B):
                        emit_qkv(4 * b)
                        emit_attn(b, 0, par)
                    for b in range(B):
                        emit_qkv(4 * b + 1)
                        emit_qkv(4 * b + 2)
                        emit_attn(b, 2, par)
                    fire_a2a(0, par)
                    # phase 1 attention: qw 1 (no QKV needed), then qw 3,
                    # with phase-0 projection interleaved into the qw-3
                    # groups.  ga(0) is fetched only after ~60us of attention
                    # has covered the collective, so its semaphore wait can't
                    # head-of-line-block the SP DMA queue.
                    for b in range(B):
                        emit_attn(b, 1, par)
                    proj0_chunks = {1: [0, 1], 2: [2], 3: [3]}
                    for b in range(B):
                        emit_qkv(4 * b + 3)
                        emit_attn(b, 3, par)
                        if b == 0:
                            emit_ga(0, par)
                        emit_proj(0, proj0_chunks.get(b, []))
                    fire_a2a(1, par)
                    emit_ga(1, par)
                    emit_proj(1, list(range(HTOK // P)))
                else:
                    for tw in range(B * NQW):
                        emit_qkv(tw)
                    for b in range(B):
                        for qw in range(NQW):
                            emit_attn(b, qw, par)
                    fire_a2a(0, par)
                    emit_ga(0, par)
                    emit_proj(0, list(range(HTOK // P)))

    nc.finalize()
    return nc


def _host_inputs(x, w_attn, w_proj, Tb=T):
    import ml_dtypes
    bf16 = ml_dtypes.bfloat16
    BT = B * Tb
    xT = np.ascontiguousarray(x.reshape(BT, C).T).astype(bf16)
    wproj_bf = np.ascontiguousarray(w_proj).astype(bf16)
    rr = np.arange(P)[:, None]
    cc = np.arange(P)[None, :]
    tri = np.where(rr > cc, np.float32(MASK_NEG), np.float32(0.0)).astype(bf16)
    masks = np.concatenate([tri, tri], axis=1)   # duplicated for both heads
    ident = np.eye(P).astype(bf16)
    in_maps = []
    for g in range(NCORES):
        in_maps.append({
            "xT": xT,
            "wq": np.ascontiguousarray(w_attn[:, DH * g: DH * (g + 1)]).astype(bf16),
            "wk": np.ascontiguousarray(w_attn[:, C + DH * g: C + DH * (g + 1)]).astype(bf16),
            "wv": np.ascontiguousarray(w_attn[:, 2 * C + DH * g: 2 * C + DH * (g + 1)]).astype(bf16),
            "wproj": wproj_bf,
            "masks": masks,
            "ident": ident,
        })
    return in_maps


_NC_CACHE = {}


def kernel(x, w_attn, w_proj):
    x = np.asarray(x)
    w_attn = np.asarray(w_attn)
    w_proj = np.asarray(w_proj)
    if T not in _NC_CACHE:
        _NC_CACHE[T] = build_nc(T)
    nc = _NC_CACHE[T]
    in_maps = _host_inputs(x, w_attn, w_proj, T)
    res = run_bass_kernel_spmd(nc, in_maps, core_ids=list(range(NCORES)))
    y = np.concatenate([res.results[g]["y"] for g in range(NCORES)], axis=0)
    return y.reshape(B, T, C).astype(np.float32)
